# revision 31
# baseline (speedup 1.0000x reference)
"""Trainium2 Bass kernel for nn_ConsciousnessMonitor (histogram_binning).

kernel(**inputs) takes FULL unsharded numpy inputs, returns the full (9,)
float32 output. Shards state_history along time across 8 NeuronCores.

Per core: stream HT [2048, 4096] with an f32->fp16 cast in the SWDGE DMA
(halves SBUF-side traffic; fp16 keeps bin-edge jitter ~1e-3) and
accumulate the 8 masked-mean series on the PE (fp16, 1 cycle/row) into 3
quadrant-packed PSUM banks. Raw-sum min/max per bank (full 128-lane DVE
reduces + PE one-hot lane gather), AllReduce(max) of [max|-min], then a
per-partition affine (scale/bias replicated across quadrants via one PE
matmul), PE transposes into t-major layout, fused clamp+int cast, fp16
one-hot (DVE 2x) and fp16 PE joint histograms packed into one PSUM bank.
One DMA ships all four joints to the AllReduce(add); MI for all 4 pairs
is computed vectorized in a [40, 10] pairs-on-partitions layout using
host-supplied block-diagonal constants. sqrt/tanh are computed via
exp/ln only, so a single activation-table reload pair happens inside the
AllReduce window (none on the critical path). The differentiation branch
(Gram matrix, variance, cdist) overlaps the stream and the collective
windows.

Self-contained: shapes/sharding hardcoded; reads no sibling files.
"""
import numpy as np

import concourse.bacc as bacc
import concourse.tile as tile
import concourse.mybir as mybir
from concourse.bass_utils import run_bass_kernel_spmd

F32 = mybir.dt.float32
F16 = mybir.dt.float16
I32 = mybir.dt.int32
AX = mybir.AxisListType
OP = mybir.AluOpType
ACT = mybir.ActivationFunctionType

N_CORES = 8
T, D = 32768, 2048
TL = T // N_CORES          # 4096 time steps per core
NB = 10                    # histogram bins per axis
NPAIR = 4                  # partitions (mask pairs)
J = 2 * NPAIR              # 8 masked-mean columns
NTC = TL // 512            # 8 accumulator groups (512 t each)
NDC = D // 128             # 16 contraction chunks
NCH = TL // 128            # 32 binning chunks of 128 t
MEM = 100
SN = 10

# accumulator tcn -> (bank b, quadrant q): tcn = 3*b + q, q in {0,1,2}
ACC_MAP = [(tcn // 3, tcn % 3) for tcn in range(NTC)]

_CACHE = {}
LAST_RESULTS = None


def _build(debug=False, variant="main"):
    sim1 = variant.startswith("sim1")
    nc = bacc.Bacc("TRN2", target_bir_lowering=False, debug=False,
                   num_devices=1 if sim1 else N_CORES)
    ht = nc.dram_tensor("ht", [D, TL], F32, kind="ExternalInput").ap()
    mmat = nc.dram_tensor("mmat", [128, NDC * J], F16,
                          kind="ExternalInput").ap()
    invc = nc.dram_tensor("invc", [128, 1], F32, kind="ExternalInput").ap()
    memt = nc.dram_tensor("memt", [128, NDC * MEM], F32,
                          kind="ExternalInput").ap()
    sampt = nc.dram_tensor("sampt", [128, NDC * SN], F32,
                           kind="ExternalInput").ap()
    sel = nc.dram_tensor("sel", [128, 3 * J], F32, kind="ExternalInput").ap()
    idrep = nc.dram_tensor("idrep", [128, J], F32, kind="ExternalInput").ap()
    idrepT = nc.dram_tensor("idrepT", [J, 128], F32,
                            kind="ExternalInput").ap()
    ident = nc.dram_tensor("ident", [NB, NB], F32, kind="ExternalInput").ap()
    bd440 = nc.dram_tensor("bd440", [NPAIR, NPAIR * NB], F32,
                           kind="ExternalInput").ap()
    bd404 = nc.dram_tensor("bd404", [NPAIR * NB, NPAIR], F32,
                           kind="ExternalInput").ap()
    out = nc.dram_tensor("out", [9], F32, kind="ExternalOutput").ap()
    if debug:
        dbg_gmm = nc.dram_tensor("dbg_gmm", [J, 2], F32,
                                 kind="ExternalOutput").ap()
        dbg_rmat = nc.dram_tensor("dbg_rmat", [128, J], F32,
                                  kind="ExternalOutput").ap()
        dbg_bin = nc.dram_tensor("dbg_bin", [128, 16], I32,
                                 kind="ExternalOutput").ap()
        dbg_gj = nc.dram_tensor("dbg_gj", [NPAIR * NB, NB], F32,
                                kind="ExternalOutput").ap()
        dbg_mm6 = nc.dram_tensor("dbg_mm6", [J, 6], F32,
                                 kind="ExternalOutput").ap()

    rg = [list(range(N_CORES))]

    with tile.TileContext(nc) as tc:
        with tc.tile_pool(name="consts", bufs=1) as consts, \
             tc.tile_pool(name="sb", bufs=1) as sb, \
             tc.tile_pool(name="htp", bufs=4) as htp, \
             tc.tile_pool(name="psA", bufs=3, space="PSUM") as psA_pool, \
             tc.tile_pool(name="psJ", bufs=2, space="PSUM") as psJ_pool, \
             tc.tile_pool(name="misc", bufs=3, space="PSUM") as misc, \
             tc.tile_pool(name="dram", bufs=1, space="DRAM") as dram:

            # ---- on-chip constants (DVE; keep Pool queue free for stream) --
            ones128 = consts.tile([128, 1], F32, tag="o128")
            nc.vector.memset(ones128[:], 1.0)
            ones10 = consts.tile([NB, 1], F32, tag="o10")
            nc.vector.memset(ones10[:], 1.0)
            ones1_10 = consts.tile([1, NB], F32, tag="o110")
            nc.vector.memset(ones1_10[:], 1.0)
            invc10 = sb.tile([128, 1], F32, tag="invc10")
            # pin the {ln, exp, copy} activation table before any ACT op
            lnscr = sb.tile([1, 1], F32, tag="lnscr")
            nc.scalar.activation(lnscr[:], ones128[0:1, :], ACT.Ln)

            # ---- early input loads (HWDGE; host pre-swizzled layouts) ----
            m_sb = consts.tile([128, NDC * J], F16, tag="msb")
            nc.sync.dma_start(out=m_sb[:], in_=mmat[:])
            invc_sb = consts.tile([128, 1], F32, tag="invc")
            nc.sync.dma_start(out=invc_sb[:], in_=invc[:])
            nc.vector.tensor_scalar(invc10[:], invc_sb[:], 10.0, None,
                                    OP.mult)
            samp_sb = consts.tile([128, NDC * SN], F32, tag="sampsb")
            nc.sync.dma_start(out=samp_sb[:], in_=sampt[:])

            # ---- stage A: stream HT (f32->fp16 cast DMA), S.T = M.T @ HT --
            psA = [psA_pool.tile([128, 512], F32, tag="sacc", name=f"psA{i}")
                   for i in range(3)]
            # clear stale PSUM rows (gather/reduce read all 128 lanes)
            for b in range(3):
                nc.vector.memset(psA[b][:], 0.0)
            for dk in range(NDC):
                htt = htp.tile([128, TL], F16, tag="htt", name="htt")
                if dk < NDC - 1:
                    halves = [(0, 128)]
                else:
                    halves = [(0, 64), (64, 128)]
                for (r0, r1) in halves:
                    nc.gpsimd.dma_start(
                        out=htt[r0:r1, :],
                        in_=ht[dk * 128 + r0:dk * 128 + r1, :])
                    for tcn in range(NTC):
                        b, q = ACC_MAP[tcn]
                        nc.tensor.matmul(psA[b][32 * q:32 * q + J, :],
                                         m_sb[r0:r1, dk * J:(dk + 1) * J],
                                         htt[r0:r1,
                                             tcn * 512:(tcn + 1) * 512],
                                         start=(dk == 0 and r0 == 0),
                                         stop=(dk == NDC - 1 and r1 == 128))

            # ---- remaining small loads ----
            mem_sb = consts.tile([128, NDC * MEM], F32, tag="memsb")
            nc.sync.dma_start(out=mem_sb[:], in_=memt[:])
            ident10 = consts.tile([NB, NB], F32, tag="id10")
            nc.sync.dma_start(out=ident10[:], in_=ident[:])
            sel_sb = consts.tile([128, 3 * J], F32, tag="selsb")
            nc.sync.dma_start(out=sel_sb[:], in_=sel[:])
            idrep_sb = consts.tile([128, J], F32, tag="idrepsb")
            nc.sync.dma_start(out=idrep_sb[:], in_=idrep[:])
            idrepT_sb = consts.tile([J, 128], F32, tag="idrepTsb")
            nc.sync.dma_start(out=idrepT_sb[:], in_=idrepT[:])
            bd440_sb = consts.tile([NPAIR, NPAIR * NB], F32, tag="bd440sb")
            nc.sync.dma_start(out=bd440_sb[:], in_=bd440[:])
            bd404_sb = consts.tile([NPAIR * NB, NPAIR], F32, tag="bd404sb")
            nc.sync.dma_start(out=bd404_sb[:], in_=bd404[:])

            # ---- differentiation branch (overlaps stream) ----
            psG = misc.tile([SN, SN], F32, tag="m", name="psG")
            for k in range(NDC):
                nc.tensor.matmul(psG[:], samp_sb[:, k * SN:(k + 1) * SN],
                                 samp_sb[:, k * SN:(k + 1) * SN],
                                 start=(k == 0), stop=(k == NDC - 1))
            sqs = sb.tile([128, NDC * SN], F32, tag="sqs")
            nc.vector.tensor_tensor(sqs[:], samp_sb[:], samp_sb[:], OP.mult)
            psr = misc.tile([SN, 1], F32, tag="m", name="psr")
            for k in range(NDC):
                nc.tensor.matmul(psr[:], sqs[:, k * SN:(k + 1) * SN],
                                 ones128[:], start=(k == 0),
                                 stop=(k == NDC - 1))
            g_sb = sb.tile([SN, SN], F32, tag="gsb")
            nc.scalar.copy(g_sb[:], psG[:])
            r_sb = sb.tile([SN, 1], F32, tag="rsb")
            nc.scalar.copy(r_sb[:], psr[:])

            # variance branch (DVE; overlaps stream)
            mem3 = mem_sb[:].rearrange("p (k f) -> p k f", f=MEM)
            mean16 = sb.tile([128, NDC], F32, tag="mean16")
            nc.vector.tensor_reduce(mean16[:], mem3, AX.X, OP.add)
            nc.vector.tensor_scalar(mean16[:], mean16[:], 1.0 / MEM, None,
                                    OP.mult)
            cent = sb.tile([128, NDC * MEM], F32, tag="cent")
            nc.vector.tensor_tensor(
                cent[:].rearrange("p (k f) -> p k f", f=MEM), mem3,
                mean16[:, :, None].broadcast_to([128, NDC, MEM]), OP.subtract)
            nc.vector.tensor_tensor(cent[:], cent[:], cent[:], OP.mult)
            var16 = sb.tile([128, NDC], F32, tag="var16")
            nc.vector.tensor_reduce(
                var16[:], cent[:].rearrange("p (k f) -> p k f", f=MEM),
                AX.X, OP.add)
            nc.vector.tensor_scalar(var16[:], var16[:], 1.0 / (MEM - 1), None,
                                    OP.mult)
            redv = sb.tile([128, 1], F32, tag="redv")
            nc.vector.tensor_reduce(redv[:], var16[:], AX.X, OP.add)
            v2 = sb.tile([128, NDC], F32, tag="v2")
            nc.vector.tensor_tensor(v2[:], var16[:], var16[:], OP.mult)
            redv2 = sb.tile([128, 1], F32, tag="redv2")
            nc.vector.tensor_reduce(redv2[:], v2[:], AX.X, OP.add)
            pstv = misc.tile([1, 1], F32, tag="m", name="pstv")
            nc.tensor.matmul(pstv[:], redv[:], ones128[:], start=True,
                             stop=True)
            tv_sb = sb.tile([1, 1], F32, tag="tvsb")
            nc.scalar.copy(tv_sb[:], pstv[:])
            pss2 = misc.tile([1, 1], F32, tag="m", name="pss2")
            nc.tensor.matmul(pss2[:], redv2[:], ones128[:], start=True,
                             stop=True)
            s2_sb = sb.tile([1, 1], F32, tag="s2sb")
            nc.scalar.copy(s2_sb[:], pss2[:])

            # cdist pieces that only need PE/copies (overlap stream)
            rrow_ps = misc.tile([1, SN], F32, tag="m", name="rrow_ps")
            nc.tensor.transpose(rrow_ps[:], r_sb[:], ident10[:])
            rrow = sb.tile([1, SN], F32, tag="rrow")
            nc.scalar.copy(rrow[:], rrow_ps[:])
            rB_ps = misc.tile([SN, SN], F32, tag="m", name="rB_ps")
            nc.tensor.matmul(rB_ps[:], ones1_10[:], rrow[:], start=True,
                             stop=True)
            rB = sb.tile([SN, SN], F32, tag="rB")
            nc.scalar.copy(rB[:], rB_ps[:])

            # ---- stage B: raw min/max per bank, PE lane-gather, AllReduce --
            stS = sb.tile([128, 3 * 512], F32, tag="sts")
            mm6 = sb.tile([128, 6], F32, tag="mm6")
            for b in range(3):
                nc.vector.tensor_reduce(mm6[:, b:b + 1], psA[b][:], AX.X,
                                        OP.max)
                nc.vector.tensor_reduce(mm6[:, 3 + b:4 + b], psA[b][:], AX.X,
                                        OP.min)
                nc.scalar.copy(stS[:, b * 512:(b + 1) * 512], psA[b][:])
            psMM = misc.tile([J, 18], F32, tag="m", name="psMM")
            for q in range(3):
                nc.tensor.matmul(psMM[:, q * 6:(q + 1) * 6],
                                 sel_sb[:, q * J:(q + 1) * J], mm6[:],
                                 start=True, stop=True)
            psMMv = psMM[:].rearrange("j (q s) -> j q s", s=6)
            mmq = sb.tile([J, 6], F32, tag="mmq")
            nc.vector.tensor_reduce(mmq[:, 0:3, None],
                                    psMMv[:, :, 0:3].rearrange(
                                        "j q s -> j s q"), AX.X, OP.max)
            nc.vector.tensor_reduce(mmq[:, 3:6, None],
                                    psMMv[:, :, 3:6].rearrange(
                                        "j q s -> j s q"), AX.X, OP.min)
            minmax = sb.tile([J, 2], F32, tag="minmax")
            nc.vector.tensor_reduce(minmax[:, 0:1], mmq[:, 0:3], AX.X,
                                    OP.max)
            tmn = sb.tile([J, 1], F32, tag="tmn")
            nc.vector.tensor_reduce(tmn[:], mmq[:, 3:6], AX.X, OP.min)
            nc.vector.tensor_scalar(minmax[:, 1:2], tmn[:], -1.0, None,
                                    OP.mult)
            cbA = dram.tile([J, 2], F32, tag="cba")
            cbB = dram.tile([J, 2], F32, tag="cbb")
            nc.sync.dma_start(out=cbA[:], in_=minmax[:])
            if sim1:
                nc.sync.dma_start(out=cbB[:], in_=cbA[:])
            else:
                nc.gpsimd.collective_compute("AllReduce", OP.max,
                                             replica_groups=rg,
                                             ins=[cbA.opt()],
                                             outs=[cbB.opt()])
            # ---- differentiation tail (fills the AllReduce windows) ----
            tvsq = sb.tile([1, 1], F32, tag="tvsq")
            nc.vector.tensor_tensor(tvsq[:], tv_sb[:], tv_sb[:], OP.mult)
            dden = sb.tile([1, 1], F32, tag="dden")
            nc.vector.scalar_tensor_tensor(dden[:], tvsq[:], 1e-6, s2_sb[:],
                                           OP.mult, OP.add)
            rdden = sb.tile([1, 1], F32, tag="rdden")
            nc.vector.reciprocal(rdden[:], dden[:])
            eff_sb = sb.tile([1, 1], F32, tag="effsb")
            nc.vector.tensor_tensor(eff_sb[:], tvsq[:], rdden[:], OP.mult)
            d2 = sb.tile([SN, SN], F32, tag="d2")
            nc.vector.scalar_tensor_tensor(d2[:], g_sb[:], -2.0, rB[:],
                                           OP.mult, OP.add)
            nc.vector.tensor_scalar(d2[:], d2[:], r_sb[:], 0.0, OP.add,
                                    OP.max)
            # sqrt(x) = exp(0.5*ln(x)); Ln ops first, then Exp ops (one
            # table switch each way, both inside the AllReduce window)
            lnd2 = sb.tile([SN, SN], F32, tag="lnd2")
            nc.scalar.activation(lnd2[:], d2[:], ACT.Ln)
            lntv = sb.tile([1, 1], F32, tag="lntv")
            nc.scalar.activation(lntv[:], tv_sb[:], ACT.Ln)
            dst = sb.tile([SN, SN], F32, tag="dst")
            nc.scalar.activation(dst[:], lnd2[:], ACT.Exp, scale=0.5)
            sqtv = sb.tile([1, 1], F32, tag="sqtv")
            nc.scalar.activation(sqtv[:], lntv[:], ACT.Exp, scale=0.5)
            dsum = sb.tile([SN, 1], F32, tag="dsum")
            nc.vector.tensor_reduce(dsum[:], dst[:], AX.X, OP.add)
            psD = misc.tile([1, 1], F32, tag="m", name="psD")
            nc.tensor.matmul(psD[:], dsum[:], ones10[:], start=True, stop=True)
            avg_sb = sb.tile([1, 1], F32, tag="avgsb")
            nc.vector.tensor_scalar(avg_sb[:], psD[:],
                                    float(1.0 / (SN * (SN - 1) + 1e-6)), None,
                                    OP.mult)
            diff_sb = sb.tile([1, 1], F32, tag="diffsb")
            nc.vector.tensor_tensor(diff_sb[:], sqtv[:], avg_sb[:], OP.mult)
            # tanh(x) = 1 - 2/(exp(2x)+1)
            e2x = sb.tile([1, 1], F32, tag="e2x")
            nc.scalar.activation(e2x[:], diff_sb[:], ACT.Exp, scale=2.0)
            nc.vector.tensor_scalar(e2x[:], e2x[:], 1.0, None, OP.add)
            re2 = sb.tile([1, 1], F32, tag="re2")
            nc.vector.reciprocal(re2[:], e2x[:])
            tanhd = sb.tile([1, 1], F32, tag="tanhd")
            nc.vector.tensor_scalar(tanhd[:], re2[:], -2.0, 1.0, OP.mult,
                                    OP.add)
            # re-pin the natural-log table before the MI log; reading tanhd
            # chains this after the last Exp so Tile cannot hoist it
            nc.scalar.activation(lnscr[:], tanhd[:], ACT.Abs)
            nc.scalar.activation(lnscr[:], lnscr[:], ACT.Ln)

            gmm = sb.tile([J, 2], F32, tag="gmm")
            nc.sync.dma_start(out=gmm[:], in_=cbB[:])

            # s1' = 10*invc/((max-min)*invc + 1e-6);
            # b1 = -min*s1' - 0.5 (RNE cast -> floor)  [raw-sum domain]
            dden2 = sb.tile([J, 1], F32, tag="dden2")
            nc.vector.tensor_tensor(dden2[:], gmm[:, 0:1], gmm[:, 1:2],
                                    OP.add)
            nc.vector.tensor_scalar(dden2[:], dden2[:], invc_sb[0:J, :], 1e-6,
                                    OP.mult, OP.add)
            rdd = sb.tile([J, 1], F32, tag="rdd")
            nc.vector.reciprocal(rdd[:], dden2[:])
            s1p8 = sb.tile([J, 2], F32, tag="s1p8")
            nc.vector.tensor_tensor(s1p8[:, 0:1], rdd[:], invc10[0:J, :],
                                    OP.mult)
            nc.vector.tensor_scalar(s1p8[:, 1:2], gmm[:, 1:2], s1p8[:, 0:1],
                                    -0.5, OP.mult, OP.add)
            # replicate [s1p | b1] to all 128 partitions via PE
            sb128_ps = misc.tile([128, 2], F32, tag="m", name="sb128")
            nc.tensor.matmul(sb128_ps[:], idrepT_sb[:], s1p8[:], start=True,
                             stop=True)
            sb128 = sb.tile([128, 2], F32, tag="sb128")
            nc.vector.tensor_copy(sb128[:], sb128_ps[:])
            s1p = sb128[:, 0:1]
            b1 = sb128[:, 1:2]

            # ---- stage C: per-partition affine, PE transpose, bin, joints --
            stSb = sb.tile([128, 3 * 512], F32, tag="stsb")
            psC = misc.tile([128, NCH * J], F32, tag="m", name="psC")
            for b in range(3):
                if b == 1:
                    nc.vector.tensor_scalar(stSb[:, b * 512:(b + 1) * 512],
                                            stS[:, b * 512:(b + 1) * 512],
                                            s1p, b1, OP.mult, OP.add)
                else:
                    nc.scalar.activation(stSb[:, b * 512:(b + 1) * 512],
                                         stS[:, b * 512:(b + 1) * 512],
                                         ACT.Identity, bias=b1,
                                         scale=s1p)
                for tcn in range(3 * b, min(3 * b + 3, NTC)):
                    _, q = ACC_MAP[tcn]
                    for c in range(4):
                        gc = tcn * 4 + c
                        nc.tensor.transpose(
                            psC[:, gc * J:(gc + 1) * J],
                            stSb[32 * q:32 * q + J,
                                 b * 512 + c * 128:b * 512 + c * 128 + 128],
                            idrep_sb[32 * q:32 * q + J, :])
            binint = sb.tile([128, NCH * J], I32, tag="binint")
            nc.vector.tensor_scalar(binint[:], psC[:], 0.0, float(NB - 1),
                                    OP.max, OP.min)
            binh = sb.tile([128, NCH * J], F16, tag="binh")
            nc.vector.tensor_copy(binh[:], binint[:])
            # one-hot bin-major: plane b at cols [b*256, (b+1)*256)
            ohsb = sb.tile([128, NB * NCH * J], F16, tag="ohsb")
            for b in range(NB):
                nc.vector.tensor_scalar(
                    ohsb[:, b * NCH * J:(b + 1) * NCH * J], binh[:],
                    float(b), None, OP.is_equal)
            ohb = ohsb[:].rearrange("p (b c) -> p b c", b=NB)
            # joint histograms packed in one PSUM bank: pairs 0-2 at rows
            # 32p cols 0:10, pair 3 at rows 0-9 cols 10:20
            psJt = psJ_pool.tile([128, 2 * NB], F32, tag="pj", name="psJt")
            nc.vector.memset(psJt[:], 0.0)
            jm1 = sb.tile([128, 2 * NB], F32, tag="jm1")
            cbj = dram.tile([96, 2 * NB], F32, tag="cbj")
            cbj2 = dram.tile([96, 2 * NB], F32, tag="cbj2")
            for p in (3, 0, 1, 2):
                outap = (psJt[0:NB, NB:2 * NB] if p == 3
                         else psJt[32 * p:32 * p + NB, 0:NB])
                for c in range(NCH):
                    nc.tensor.matmul(outap,
                                     ohb[:, :, c * J + 2 * p],
                                     ohb[:, :, c * J + 2 * p + 1],
                                     start=(c == 0), stop=(c == NCH - 1))
            nc.scalar.copy(jm1[:], psJt[:])
            nc.sync.dma_start(out=cbj[:], in_=jm1[0:96, :])
            if sim1:
                nc.sync.dma_start(out=cbj2[:], in_=cbj[:])
            else:
                nc.gpsimd.collective_compute("AllReduce", OP.add,
                                             replica_groups=rg,
                                             ins=[cbj.opt()],
                                             outs=[cbj2.opt()])
            # gj4: pairs stacked along partitions, [40, 10]
            gj4 = sb.tile([NPAIR * NB, NB], F32, tag="gj4")
            nc.sync.dma_start(
                out=gj4[0:3 * NB, :],
                in_=cbj2[:].rearrange("(q r) c -> q r c",
                                      r=32)[0:3, 0:NB, 0:NB])
            nc.scalar.dma_start(out=gj4[3 * NB:4 * NB, :],
                                in_=cbj2[0:NB, NB:2 * NB])

            # ---- stage D: MI for all 4 pairs at once ----
            rowsum = sb.tile([NPAIR * NB, 1], F32, tag="rowsum")
            nc.vector.tensor_reduce(rowsum[:], gj4[:], AX.X, OP.add)
            colps = misc.tile([NPAIR, NB], F32, tag="m", name="colps")
            nc.tensor.matmul(colps[:], bd404_sb[:], gj4[:], start=True,
                             stop=True)
            tot4 = sb.tile([NPAIR, 1], F32, tag="tot4")
            nc.vector.tensor_reduce(tot4[:], colps[:], AX.X, OP.add)
            nc.vector.tensor_scalar(tot4[:], tot4[:], 1e-10, None, OP.add)
            tp = sb.tile([NPAIR, 1 + NB], F32, tag="tp")
            nc.vector.reciprocal(tp[:, 0:1], tot4[:])
            nc.vector.tensor_scalar(tp[:, 1:1 + NB], colps[:], tp[:, 0:1],
                                    None, OP.mult)
            tpB = misc.tile([NPAIR * NB, 1 + NB], F32, tag="m", name="tpB")
            nc.tensor.matmul(tpB[:], bd440_sb[:], tp[:], start=True,
                             stop=True)
            t40 = sb.tile([NPAIR * NB, 1], F32, tag="t40")
            nc.scalar.copy(t40[:], tpB[:, 0:1])
            pyB = tpB[:, 1:1 + NB]
            px = sb.tile([NPAIR * NB, 1], F32, tag="px")
            nc.vector.tensor_tensor(px[:], rowsum[:], t40[:], OP.mult)
            jn = sb.tile([NPAIR * NB, NB], F32, tag="jn")
            nc.vector.tensor_scalar(jn[:], gj4[:], t40[:], None, OP.mult)
            num = sb.tile([NPAIR * NB, NB], F32, tag="num")
            nc.vector.tensor_scalar(num[:], jn[:], 1e-10, None, OP.add)
            outer = sb.tile([NPAIR * NB, NB], F32, tag="outer")
            nc.vector.tensor_scalar(outer[:], pyB, px[:], 1e-10, OP.mult,
                                    OP.add)
            rout = sb.tile([NPAIR * NB, NB], F32, tag="rout")
            nc.vector.reciprocal(rout[:], outer[:])
            nc.vector.tensor_tensor(num[:], num[:], rout[:], OP.mult)
            lg = sb.tile([NPAIR * NB, NB], F32, tag="lg")
            nc.scalar.activation(lg[:], num[:], ACT.Ln)
            nc.vector.tensor_tensor(lg[:], jn[:], lg[:], OP.mult)
            ms = sb.tile([NPAIR * NB, 1], F32, tag="ms")
            nc.vector.tensor_reduce(ms[:], lg[:], AX.X, OP.add)
            mi4_ps = misc.tile([NPAIR, 1], F32, tag="m", name="mi4_ps")
            nc.tensor.matmul(mi4_ps[:], bd404_sb[:], ms[:], start=True,
                             stop=True)
            mi4 = sb.tile([NPAIR, 1], F32, tag="mi4")
            nc.vector.tensor_scalar(mi4[:], mi4_ps[:], 0.0, None, OP.max)
            mit_ps = misc.tile([1, NPAIR], F32, tag="m", name="mit_ps")
            nc.tensor.transpose(mit_ps[:], mi4[:], ident10[0:NPAIR, 0:NPAIR])
            outrow = sb.tile([1, 9], F32, tag="outrow")
            nc.vector.tensor_copy(outrow[:, 1:2], diff_sb[:])
            nc.vector.tensor_copy(outrow[:, 2:3], eff_sb[:])
            nc.vector.tensor_copy(outrow[:, 3:4], tv_sb[:])
            nc.scalar.copy(outrow[:, 5:9], mit_ps[:])
            nc.vector.tensor_reduce(outrow[:, 4:5], outrow[:, 5:9], AX.X,
                                    OP.min)
            nc.vector.tensor_tensor(outrow[:, 0:1], outrow[:, 4:5], tanhd[:],
                                    OP.add)
            nc.sync.dma_start(out=out[:], in_=outrow[:])
            if debug:
                nc.sync.dma_start(out=dbg_gmm[:], in_=gmm[:])
                nc.sync.dma_start(out=dbg_rmat[:], in_=stSb[:, 0:J])
                nc.sync.dma_start(out=dbg_bin[:], in_=binint[:, 0:16])
                nc.sync.dma_start(out=dbg_gj[:], in_=gj4[:])
                nc.sync.dma_start(out=dbg_mm6[:], in_=mmq[:])

    nc.compile()
    return nc


def _get_nc(debug=False):
    key = ("ncd" if debug else "nc")
    if key not in _CACHE:
        _CACHE[key] = _build(debug)
    return _CACHE[key]


def kernel(state, state_memory, state_history, partitions, sample_idx,
           trace=False, debug=False):
    global LAST_RESULTS
    state = np.asarray(state, np.float32)
    state_memory = np.asarray(state_memory, np.float32)
    state_history = np.asarray(state_history, np.float32)
    partitions = np.asarray(partitions)
    sample_idx = np.asarray(sample_idx)

    pf = partitions.astype(np.float32)
    mmat = np.empty((D, J), np.float32)
    invc8 = np.empty((J,), np.float32)
    for p in range(NPAIR):
        mmat[:, 2 * p] = pf[p]
        mmat[:, 2 * p + 1] = np.float32(1.0) - pf[p]
        invc8[2 * p] = np.float32(1.0) / pf[p].sum(dtype=np.float32)
        invc8[2 * p + 1] = np.float32(1.0) / (np.float32(1.0) - pf[p]).sum(
            dtype=np.float32)
    invc = np.zeros((128, 1), np.float32)
    for q in range(3):
        invc[32 * q:32 * q + J, 0] = invc8

    # SBUF-layout pre-swizzles: [p, k*F + f] = src[k*128 + p, f]
    def swz(src_dxf):
        f = src_dxf.shape[1]
        return np.ascontiguousarray(
            src_dxf.reshape(NDC, 128, f).transpose(1, 0, 2).reshape(
                128, NDC * f))

    mmatb = swz(mmat).astype(np.float16)
    memory = np.concatenate([state, state_memory[state.shape[0]:]], axis=0)
    memt = swz(np.ascontiguousarray(memory.T))
    sampt = swz(np.ascontiguousarray(memory[sample_idx].T))

    sel = np.zeros((128, 3 * J), np.float32)
    idrep = np.zeros((128, J), np.float32)
    for q in range(3):
        for j in range(J):
            sel[32 * q + j, q * J + j] = 1.0
            idrep[32 * q + j, j] = 1.0
    idrepT = np.ascontiguousarray(idrep.T)
    ident = np.eye(NB, dtype=np.float32)
    bd440 = np.zeros((NPAIR, NPAIR * NB), np.float32)
    bd404 = np.zeros((NPAIR * NB, NPAIR), np.float32)
    for p in range(NPAIR):
        bd440[p, p * NB:(p + 1) * NB] = 1.0
        bd404[p * NB:(p + 1) * NB, p] = 1.0

    in_maps = []
    for c in range(N_CORES):
        htc = np.ascontiguousarray(state_history[c * TL:(c + 1) * TL, :].T)
        in_maps.append({"ht": htc, "mmat": mmatb, "invc": invc,
                        "memt": memt, "sampt": sampt, "sel": sel,
                        "idrep": idrep, "idrepT": idrepT, "ident": ident,
                        "bd440": bd440,
                        "bd404": bd404})

    nc = _get_nc(debug)
    res = run_bass_kernel_spmd(nc, in_maps, list(range(N_CORES)),
                               trace=trace)
    LAST_RESULTS = res
    return np.asarray(res.results[0]["out"], np.float32)


# revision 33
# speedup vs baseline: 1.0060x; 1.0060x over previous
"""Trainium2 Bass kernel for nn_ConsciousnessMonitor (histogram_binning).

kernel(**inputs) takes FULL unsharded numpy inputs, returns the full (9,)
float32 output. Shards state_history along time across 8 NeuronCores.

Per core: stream HT [2048, 4096] with an f32->fp16 cast in the SWDGE DMA
(halves SBUF-side traffic; fp16 keeps bin-edge jitter ~1e-3) and
accumulate the 8 masked-mean series on the PE (fp16, 1 cycle/row) into 3
quadrant-packed PSUM banks. Raw-sum min/max per bank (full 128-lane DVE
reduces + PE one-hot lane gather), AllReduce(max) of [max|-min], then a
per-partition affine (scale/bias replicated across quadrants via one PE
matmul), PE transposes into t-major layout, fused clamp+int cast, fp16
one-hot (DVE 2x) and fp16 PE joint histograms packed into one PSUM bank.
One DMA ships all four joints to the AllReduce(add); MI for all 4 pairs
is computed vectorized in a [40, 10] pairs-on-partitions layout using
host-supplied block-diagonal constants. sqrt/tanh are computed via
exp/ln only, so a single activation-table reload pair happens inside the
AllReduce window (none on the critical path). The differentiation branch
(Gram matrix, variance, cdist) overlaps the stream and the collective
windows.

Self-contained: shapes/sharding hardcoded; reads no sibling files.
"""
import numpy as np

import concourse.bacc as bacc
import concourse.tile as tile
import concourse.mybir as mybir
from concourse.bass_utils import run_bass_kernel_spmd

F32 = mybir.dt.float32
F16 = mybir.dt.float16
I32 = mybir.dt.int32
AX = mybir.AxisListType
OP = mybir.AluOpType
ACT = mybir.ActivationFunctionType

N_CORES = 8
T, D = 32768, 2048
TL = T // N_CORES          # 4096 time steps per core
NB = 10                    # histogram bins per axis
NPAIR = 4                  # partitions (mask pairs)
J = 2 * NPAIR              # 8 masked-mean columns
NTC = TL // 512            # 8 accumulator groups (512 t each)
NDC = D // 128             # 16 contraction chunks
NCH = TL // 128            # 32 binning chunks of 128 t
MEM = 100
SN = 10

# accumulator tcn -> (bank b, quadrant q): tcn = 3*b + q, q in {0,1,2}
ACC_MAP = [(tcn // 3, tcn % 3) for tcn in range(NTC)]

_CACHE = {}
LAST_RESULTS = None


def _build(debug=False, variant="main"):
    sim1 = variant.startswith("sim1")
    nc = bacc.Bacc("TRN2", target_bir_lowering=False, debug=False,
                   num_devices=1 if sim1 else N_CORES)
    ht = nc.dram_tensor("ht", [D, TL], F32, kind="ExternalInput").ap()
    mmat = nc.dram_tensor("mmat", [128, NDC * J], F16,
                          kind="ExternalInput").ap()
    invc = nc.dram_tensor("invc", [128, 1], F32, kind="ExternalInput").ap()
    memt = nc.dram_tensor("memt", [128, NDC * MEM], F32,
                          kind="ExternalInput").ap()
    sampt = nc.dram_tensor("sampt", [128, NDC * SN], F32,
                           kind="ExternalInput").ap()
    sel = nc.dram_tensor("sel", [128, 3 * J], F32, kind="ExternalInput").ap()
    idrep = nc.dram_tensor("idrep", [128, J], F32, kind="ExternalInput").ap()
    idrepT = nc.dram_tensor("idrepT", [J, 128], F32,
                            kind="ExternalInput").ap()
    ident = nc.dram_tensor("ident", [NB, NB], F32, kind="ExternalInput").ap()
    bd440 = nc.dram_tensor("bd440", [NPAIR, NPAIR * NB], F32,
                           kind="ExternalInput").ap()
    bd404 = nc.dram_tensor("bd404", [NPAIR * NB, NPAIR], F32,
                           kind="ExternalInput").ap()
    out = nc.dram_tensor("out", [9], F32, kind="ExternalOutput").ap()
    if debug:
        dbg_gmm = nc.dram_tensor("dbg_gmm", [J, 2], F32,
                                 kind="ExternalOutput").ap()
        dbg_rmat = nc.dram_tensor("dbg_rmat", [128, J], F32,
                                  kind="ExternalOutput").ap()
        dbg_bin = nc.dram_tensor("dbg_bin", [128, 16], I32,
                                 kind="ExternalOutput").ap()
        dbg_gj = nc.dram_tensor("dbg_gj", [NPAIR * NB, NB], F32,
                                kind="ExternalOutput").ap()
        dbg_mm6 = nc.dram_tensor("dbg_mm6", [J, 6], F32,
                                 kind="ExternalOutput").ap()

    rg = [list(range(N_CORES))]

    with tile.TileContext(nc) as tc:
        with tc.tile_pool(name="consts", bufs=1) as consts, \
             tc.tile_pool(name="sb", bufs=1) as sb, \
             tc.tile_pool(name="htp", bufs=4) as htp, \
             tc.tile_pool(name="psA", bufs=3, space="PSUM") as psA_pool, \
             tc.tile_pool(name="psJ", bufs=2, space="PSUM") as psJ_pool, \
             tc.tile_pool(name="misc", bufs=3, space="PSUM") as misc, \
             tc.tile_pool(name="dram", bufs=1, space="DRAM") as dram:

            # ---- on-chip constants (DVE; keep Pool queue free for stream) --
            ones128 = consts.tile([128, 1], F32, tag="o128")
            nc.vector.memset(ones128[:], 1.0)
            ones10 = consts.tile([NB, 1], F32, tag="o10")
            nc.vector.memset(ones10[:], 1.0)
            ones1_10 = consts.tile([1, NB], F32, tag="o110")
            nc.vector.memset(ones1_10[:], 1.0)
            invc10 = sb.tile([128, 1], F32, tag="invc10")
            # pin the {ln, exp, copy} activation table before any ACT op
            lnscr = sb.tile([1, 1], F32, tag="lnscr")
            nc.scalar.activation(lnscr[:], ones128[0:1, :], ACT.Ln)

            # ---- early input loads (HWDGE; host pre-swizzled layouts) ----
            m_sb = consts.tile([128, NDC * J], F16, tag="msb")
            nc.sync.dma_start(out=m_sb[:], in_=mmat[:])
            invc_sb = consts.tile([128, 1], F32, tag="invc")
            nc.sync.dma_start(out=invc_sb[:], in_=invc[:])
            nc.vector.tensor_scalar(invc10[:], invc_sb[:], 10.0, None,
                                    OP.mult)
            samp_sb = consts.tile([128, NDC * SN], F32, tag="sampsb")
            nc.sync.dma_start(out=samp_sb[:], in_=sampt[:])

            # ---- stage A: stream HT (f32->fp16 cast DMA), S.T = M.T @ HT --
            psA = [psA_pool.tile([128, 512], F32, tag="sacc", name=f"psA{i}")
                   for i in range(3)]
            # clear stale PSUM rows (gather/reduce read all 128 lanes)
            for b in range(3):
                nc.vector.memset(psA[b][:], 0.0)
            for dk in range(NDC):
                htt = htp.tile([128, TL], F16, tag="htt", name="htt")
                nc.gpsimd.dma_start(out=htt[:],
                                    in_=ht[dk * 128:(dk + 1) * 128, :])
                for tcn in range(NTC):
                    b, q = ACC_MAP[tcn]
                    nc.tensor.matmul(psA[b][32 * q:32 * q + J, :],
                                     m_sb[:, dk * J:(dk + 1) * J],
                                     htt[:, tcn * 512:(tcn + 1) * 512],
                                     start=(dk == 0), stop=(dk == NDC - 1))

            # ---- remaining small loads ----
            mem_sb = consts.tile([128, NDC * MEM], F32, tag="memsb")
            nc.sync.dma_start(out=mem_sb[:], in_=memt[:])
            ident10 = consts.tile([NB, NB], F32, tag="id10")
            nc.sync.dma_start(out=ident10[:], in_=ident[:])
            sel_sb = consts.tile([128, 3 * J], F32, tag="selsb")
            nc.sync.dma_start(out=sel_sb[:], in_=sel[:])
            idrep_sb = consts.tile([128, J], F32, tag="idrepsb")
            nc.sync.dma_start(out=idrep_sb[:], in_=idrep[:])
            idrepT_sb = consts.tile([J, 128], F32, tag="idrepTsb")
            nc.sync.dma_start(out=idrepT_sb[:], in_=idrepT[:])
            bd440_sb = consts.tile([NPAIR, NPAIR * NB], F32, tag="bd440sb")
            nc.sync.dma_start(out=bd440_sb[:], in_=bd440[:])
            bd404_sb = consts.tile([NPAIR * NB, NPAIR], F32, tag="bd404sb")
            nc.sync.dma_start(out=bd404_sb[:], in_=bd404[:])

            # ---- differentiation branch (overlaps stream) ----
            psG = misc.tile([SN, SN], F32, tag="m", name="psG")
            for k in range(NDC):
                nc.tensor.matmul(psG[:], samp_sb[:, k * SN:(k + 1) * SN],
                                 samp_sb[:, k * SN:(k + 1) * SN],
                                 start=(k == 0), stop=(k == NDC - 1))
            sqs = sb.tile([128, NDC * SN], F32, tag="sqs")
            nc.vector.tensor_tensor(sqs[:], samp_sb[:], samp_sb[:], OP.mult)
            psr = misc.tile([SN, 1], F32, tag="m", name="psr")
            for k in range(NDC):
                nc.tensor.matmul(psr[:], sqs[:, k * SN:(k + 1) * SN],
                                 ones128[:], start=(k == 0),
                                 stop=(k == NDC - 1))
            g_sb = sb.tile([SN, SN], F32, tag="gsb")
            nc.scalar.copy(g_sb[:], psG[:])
            r_sb = sb.tile([SN, 1], F32, tag="rsb")
            nc.scalar.copy(r_sb[:], psr[:])

            # variance branch (DVE; overlaps stream)
            mem3 = mem_sb[:].rearrange("p (k f) -> p k f", f=MEM)
            mean16 = sb.tile([128, NDC], F32, tag="mean16")
            nc.vector.tensor_reduce(mean16[:], mem3, AX.X, OP.add)
            nc.vector.tensor_scalar(mean16[:], mean16[:], 1.0 / MEM, None,
                                    OP.mult)
            cent = sb.tile([128, NDC * MEM], F32, tag="cent")
            nc.vector.tensor_tensor(
                cent[:].rearrange("p (k f) -> p k f", f=MEM), mem3,
                mean16[:, :, None].broadcast_to([128, NDC, MEM]), OP.subtract)
            nc.vector.tensor_tensor(cent[:], cent[:], cent[:], OP.mult)
            var16 = sb.tile([128, NDC], F32, tag="var16")
            nc.vector.tensor_reduce(
                var16[:], cent[:].rearrange("p (k f) -> p k f", f=MEM),
                AX.X, OP.add)
            nc.vector.tensor_scalar(var16[:], var16[:], 1.0 / (MEM - 1), None,
                                    OP.mult)
            redv = sb.tile([128, 1], F32, tag="redv")
            nc.vector.tensor_reduce(redv[:], var16[:], AX.X, OP.add)
            v2 = sb.tile([128, NDC], F32, tag="v2")
            nc.vector.tensor_tensor(v2[:], var16[:], var16[:], OP.mult)
            redv2 = sb.tile([128, 1], F32, tag="redv2")
            nc.vector.tensor_reduce(redv2[:], v2[:], AX.X, OP.add)
            pstv = misc.tile([1, 1], F32, tag="m", name="pstv")
            nc.tensor.matmul(pstv[:], redv[:], ones128[:], start=True,
                             stop=True)
            tv_sb = sb.tile([1, 1], F32, tag="tvsb")
            nc.scalar.copy(tv_sb[:], pstv[:])
            pss2 = misc.tile([1, 1], F32, tag="m", name="pss2")
            nc.tensor.matmul(pss2[:], redv2[:], ones128[:], start=True,
                             stop=True)
            s2_sb = sb.tile([1, 1], F32, tag="s2sb")
            nc.scalar.copy(s2_sb[:], pss2[:])

            # cdist pieces that only need PE/copies (overlap stream)
            rrow_ps = misc.tile([1, SN], F32, tag="m", name="rrow_ps")
            nc.tensor.transpose(rrow_ps[:], r_sb[:], ident10[:])
            rrow = sb.tile([1, SN], F32, tag="rrow")
            nc.scalar.copy(rrow[:], rrow_ps[:])
            rB_ps = misc.tile([SN, SN], F32, tag="m", name="rB_ps")
            nc.tensor.matmul(rB_ps[:], ones1_10[:], rrow[:], start=True,
                             stop=True)
            rB = sb.tile([SN, SN], F32, tag="rB")
            nc.scalar.copy(rB[:], rB_ps[:])

            # ---- stage B: raw min/max per bank, PE lane-gather, AllReduce --
            stS = sb.tile([128, 3 * 512], F32, tag="sts")
            mm6 = sb.tile([128, 6], F32, tag="mm6")
            for b in range(3):
                nc.vector.tensor_reduce(mm6[:, b:b + 1], psA[b][:], AX.X,
                                        OP.max)
                nc.vector.tensor_reduce(mm6[:, 3 + b:4 + b], psA[b][:], AX.X,
                                        OP.min)
                nc.scalar.copy(stS[:, b * 512:(b + 1) * 512], psA[b][:])
            psMM = misc.tile([J, 18], F32, tag="m", name="psMM")
            for q in range(3):
                nc.tensor.matmul(psMM[:, q * 6:(q + 1) * 6],
                                 sel_sb[:, q * J:(q + 1) * J], mm6[:],
                                 start=True, stop=True)
            psMMv = psMM[:].rearrange("j (q s) -> j q s", s=6)
            mmq = sb.tile([J, 6], F32, tag="mmq")
            nc.vector.tensor_reduce(mmq[:, 0:3, None],
                                    psMMv[:, :, 0:3].rearrange(
                                        "j q s -> j s q"), AX.X, OP.max)
            nc.vector.tensor_reduce(mmq[:, 3:6, None],
                                    psMMv[:, :, 3:6].rearrange(
                                        "j q s -> j s q"), AX.X, OP.min)
            minmax = sb.tile([J, 2], F32, tag="minmax")
            nc.vector.tensor_reduce(minmax[:, 0:1], mmq[:, 0:3], AX.X,
                                    OP.max)
            tmn = sb.tile([J, 1], F32, tag="tmn")
            nc.vector.tensor_reduce(tmn[:], mmq[:, 3:6], AX.X, OP.min)
            nc.vector.tensor_scalar(minmax[:, 1:2], tmn[:], -1.0, None,
                                    OP.mult)
            cbA = dram.tile([J, 2], F32, tag="cba")
            cbB = dram.tile([J, 2], F32, tag="cbb")
            nc.sync.dma_start(out=cbA[:], in_=minmax[:])
            if sim1:
                nc.sync.dma_start(out=cbB[:], in_=cbA[:])
            else:
                nc.gpsimd.collective_compute("AllReduce", OP.max,
                                             replica_groups=rg,
                                             ins=[cbA.opt()],
                                             outs=[cbB.opt()])
            # ---- differentiation tail (fills the AllReduce windows) ----
            tvsq = sb.tile([1, 1], F32, tag="tvsq")
            nc.vector.tensor_tensor(tvsq[:], tv_sb[:], tv_sb[:], OP.mult)
            dden = sb.tile([1, 1], F32, tag="dden")
            nc.vector.scalar_tensor_tensor(dden[:], tvsq[:], 1e-6, s2_sb[:],
                                           OP.mult, OP.add)
            rdden = sb.tile([1, 1], F32, tag="rdden")
            nc.vector.reciprocal(rdden[:], dden[:])
            eff_sb = sb.tile([1, 1], F32, tag="effsb")
            nc.vector.tensor_tensor(eff_sb[:], tvsq[:], rdden[:], OP.mult)
            d2 = sb.tile([SN, SN], F32, tag="d2")
            nc.vector.scalar_tensor_tensor(d2[:], g_sb[:], -2.0, rB[:],
                                           OP.mult, OP.add)
            nc.vector.tensor_scalar(d2[:], d2[:], r_sb[:], 0.0, OP.add,
                                    OP.max)
            # sqrt(x) = exp(0.5*ln(x)); Ln ops first, then Exp ops (one
            # table switch each way, both inside the AllReduce window)
            lnd2 = sb.tile([SN, SN], F32, tag="lnd2")
            nc.scalar.activation(lnd2[:], d2[:], ACT.Ln)
            lntv = sb.tile([1, 1], F32, tag="lntv")
            nc.scalar.activation(lntv[:], tv_sb[:], ACT.Ln)
            dst = sb.tile([SN, SN], F32, tag="dst")
            nc.scalar.activation(dst[:], lnd2[:], ACT.Exp, scale=0.5)
            sqtv = sb.tile([1, 1], F32, tag="sqtv")
            nc.scalar.activation(sqtv[:], lntv[:], ACT.Exp, scale=0.5)
            dsum = sb.tile([SN, 1], F32, tag="dsum")
            nc.vector.tensor_reduce(dsum[:], dst[:], AX.X, OP.add)
            psD = misc.tile([1, 1], F32, tag="m", name="psD")
            nc.tensor.matmul(psD[:], dsum[:], ones10[:], start=True, stop=True)
            avg_sb = sb.tile([1, 1], F32, tag="avgsb")
            nc.vector.tensor_scalar(avg_sb[:], psD[:],
                                    float(1.0 / (SN * (SN - 1) + 1e-6)), None,
                                    OP.mult)
            diff_sb = sb.tile([1, 1], F32, tag="diffsb")
            nc.vector.tensor_tensor(diff_sb[:], sqtv[:], avg_sb[:], OP.mult)
            # tanh(x) = 1 - 2/(exp(2x)+1)
            e2x = sb.tile([1, 1], F32, tag="e2x")
            nc.scalar.activation(e2x[:], diff_sb[:], ACT.Exp, scale=2.0)
            nc.vector.tensor_scalar(e2x[:], e2x[:], 1.0, None, OP.add)
            re2 = sb.tile([1, 1], F32, tag="re2")
            nc.vector.reciprocal(re2[:], e2x[:])
            tanhd = sb.tile([1, 1], F32, tag="tanhd")
            nc.vector.tensor_scalar(tanhd[:], re2[:], -2.0, 1.0, OP.mult,
                                    OP.add)
            # re-pin the natural-log table before the MI log; reading tanhd
            # chains this after the last Exp so Tile cannot hoist it
            nc.scalar.activation(lnscr[:], tanhd[:], ACT.Abs)
            nc.scalar.activation(lnscr[:], lnscr[:], ACT.Ln)

            gmm = sb.tile([J, 2], F32, tag="gmm")
            nc.sync.dma_start(out=gmm[:], in_=cbB[:])

            # s1' = 10*invc/((max-min)*invc + 1e-6);
            # b1 = -min*s1' - 0.5 (RNE cast -> floor)  [raw-sum domain]
            dden2 = sb.tile([J, 1], F32, tag="dden2")
            nc.vector.tensor_tensor(dden2[:], gmm[:, 0:1], gmm[:, 1:2],
                                    OP.add)
            nc.vector.tensor_scalar(dden2[:], dden2[:], invc_sb[0:J, :], 1e-6,
                                    OP.mult, OP.add)
            rdd = sb.tile([J, 1], F32, tag="rdd")
            nc.vector.reciprocal(rdd[:], dden2[:])
            s1p8 = sb.tile([J, 2], F32, tag="s1p8")
            nc.vector.tensor_tensor(s1p8[:, 0:1], rdd[:], invc10[0:J, :],
                                    OP.mult)
            nc.vector.tensor_scalar(s1p8[:, 1:2], gmm[:, 1:2], s1p8[:, 0:1],
                                    -0.5, OP.mult, OP.add)
            # replicate [s1p | b1] to all 128 partitions via PE
            sb128_ps = misc.tile([128, 2], F32, tag="m", name="sb128")
            nc.tensor.matmul(sb128_ps[:], idrepT_sb[:], s1p8[:], start=True,
                             stop=True)
            sb128 = sb.tile([128, 2], F32, tag="sb128")
            nc.vector.tensor_copy(sb128[:], sb128_ps[:])
            s1p = sb128[:, 0:1]
            b1 = sb128[:, 1:2]

            # ---- stage C: per-partition affine, PE transpose, bin, joints --
            stSb = sb.tile([128, 3 * 512], F32, tag="stsb")
            psC = misc.tile([128, NCH * J], F32, tag="m", name="psC")
            for b in range(3):
                if b == 1:
                    nc.vector.tensor_scalar(stSb[:, b * 512:(b + 1) * 512],
                                            stS[:, b * 512:(b + 1) * 512],
                                            s1p, b1, OP.mult, OP.add)
                else:
                    nc.scalar.activation(stSb[:, b * 512:(b + 1) * 512],
                                         stS[:, b * 512:(b + 1) * 512],
                                         ACT.Identity, bias=b1,
                                         scale=s1p)
                for tcn in range(3 * b, min(3 * b + 3, NTC)):
                    _, q = ACC_MAP[tcn]
                    for c in range(4):
                        gc = tcn * 4 + c
                        nc.tensor.transpose(
                            psC[:, gc * J:(gc + 1) * J],
                            stSb[32 * q:32 * q + J,
                                 b * 512 + c * 128:b * 512 + c * 128 + 128],
                            idrep_sb[32 * q:32 * q + J, :])
            binint = sb.tile([128, NCH * J], I32, tag="binint")
            nc.vector.tensor_scalar(binint[:], psC[:], 0.0, float(NB - 1),
                                    OP.max, OP.min)
            binh = sb.tile([128, NCH * J], F16, tag="binh")
            nc.vector.tensor_copy(binh[:], binint[:])
            # one-hot bin-major: plane b at cols [b*256, (b+1)*256)
            ohsb = sb.tile([128, NB * NCH * J], F16, tag="ohsb")
            for b in range(NB):
                nc.vector.tensor_scalar(
                    ohsb[:, b * NCH * J:(b + 1) * NCH * J], binh[:],
                    float(b), None, OP.is_equal)
            ohb = ohsb[:].rearrange("p (b c) -> p b c", b=NB)
            # joint histograms packed in one PSUM bank: pairs 0-2 at rows
            # 32p cols 0:10, pair 3 at rows 0-9 cols 10:20
            psJt = psJ_pool.tile([128, 2 * NB], F32, tag="pj", name="psJt")
            nc.vector.memset(psJt[:], 0.0)
            jm1 = sb.tile([128, 2 * NB], F32, tag="jm1")
            cbj = dram.tile([96, 2 * NB], F32, tag="cbj")
            cbj2 = dram.tile([96, 2 * NB], F32, tag="cbj2")
            for p in (3, 0, 1, 2):
                outap = (psJt[0:NB, NB:2 * NB] if p == 3
                         else psJt[32 * p:32 * p + NB, 0:NB])
                for c in range(NCH):
                    nc.tensor.matmul(outap,
                                     ohb[:, :, c * J + 2 * p],
                                     ohb[:, :, c * J + 2 * p + 1],
                                     start=(c == 0), stop=(c == NCH - 1))
            nc.scalar.copy(jm1[:], psJt[:])
            nc.sync.dma_start(out=cbj[:], in_=jm1[0:96, :])
            if sim1:
                nc.sync.dma_start(out=cbj2[:], in_=cbj[:])
            else:
                nc.gpsimd.collective_compute("AllReduce", OP.add,
                                             replica_groups=rg,
                                             ins=[cbj.opt()],
                                             outs=[cbj2.opt()])
            # gj4: pairs stacked along partitions, [40, 10]
            gj4 = sb.tile([NPAIR * NB, NB], F32, tag="gj4")
            nc.sync.dma_start(
                out=gj4[0:3 * NB, :],
                in_=cbj2[:].rearrange("(q r) c -> q r c",
                                      r=32)[0:3, 0:NB, 0:NB])
            nc.scalar.dma_start(out=gj4[3 * NB:4 * NB, :],
                                in_=cbj2[0:NB, NB:2 * NB])

            # ---- stage D: MI for all 4 pairs at once ----
            rowsum = sb.tile([NPAIR * NB, 1], F32, tag="rowsum")
            nc.vector.tensor_reduce(rowsum[:], gj4[:], AX.X, OP.add)
            colps = misc.tile([NPAIR, NB], F32, tag="m", name="colps")
            nc.tensor.matmul(colps[:], bd404_sb[:], gj4[:], start=True,
                             stop=True)
            tot4 = sb.tile([NPAIR, 1], F32, tag="tot4")
            nc.vector.tensor_reduce(tot4[:], colps[:], AX.X, OP.add)
            nc.vector.tensor_scalar(tot4[:], tot4[:], 1e-10, None, OP.add)
            tinv = sb.tile([NPAIR, 1], F32, tag="tinv")
            nc.vector.reciprocal(tinv[:], tot4[:])
            t40_ps = misc.tile([NPAIR * NB, 1], F32, tag="m", name="t40_ps")
            nc.tensor.matmul(t40_ps[:], bd440_sb[:], tinv[:], start=True,
                             stop=True)
            t40 = sb.tile([NPAIR * NB, 1], F32, tag="t40")
            nc.scalar.copy(t40[:], t40_ps[:])
            pyn = sb.tile([NPAIR, NB], F32, tag="pyn")
            nc.vector.tensor_scalar(pyn[:], colps[:], tinv[:], None, OP.mult)
            pyB = misc.tile([NPAIR * NB, NB], F32, tag="m", name="pyB")
            nc.tensor.matmul(pyB[:], bd440_sb[:], pyn[:], start=True,
                             stop=True)
            px = sb.tile([NPAIR * NB, 1], F32, tag="px")
            nc.vector.tensor_tensor(px[:], rowsum[:], t40[:], OP.mult)
            jn = sb.tile([NPAIR * NB, NB], F32, tag="jn")
            nc.vector.tensor_scalar(jn[:], gj4[:], t40[:], None, OP.mult)
            num = sb.tile([NPAIR * NB, NB], F32, tag="num")
            nc.vector.tensor_scalar(num[:], jn[:], 1e-10, None, OP.add)
            outer = sb.tile([NPAIR * NB, NB], F32, tag="outer")
            nc.vector.tensor_scalar(outer[:], pyB[:], px[:], 1e-10, OP.mult,
                                    OP.add)
            rout = sb.tile([NPAIR * NB, NB], F32, tag="rout")
            nc.vector.reciprocal(rout[:], outer[:])
            nc.vector.tensor_tensor(num[:], num[:], rout[:], OP.mult)
            lg = sb.tile([NPAIR * NB, NB], F32, tag="lg")
            nc.scalar.activation(lg[:], num[:], ACT.Ln)
            nc.vector.tensor_tensor(lg[:], jn[:], lg[:], OP.mult)
            ms = sb.tile([NPAIR * NB, 1], F32, tag="ms")
            nc.vector.tensor_reduce(ms[:], lg[:], AX.X, OP.add)
            mi4_ps = misc.tile([NPAIR, 1], F32, tag="m", name="mi4_ps")
            nc.tensor.matmul(mi4_ps[:], bd404_sb[:], ms[:], start=True,
                             stop=True)
            mi4 = sb.tile([NPAIR, 1], F32, tag="mi4")
            nc.vector.tensor_scalar(mi4[:], mi4_ps[:], 0.0, None, OP.max)
            mit_ps = misc.tile([1, NPAIR], F32, tag="m", name="mit_ps")
            nc.tensor.transpose(mit_ps[:], mi4[:], ident10[0:NPAIR, 0:NPAIR])
            outrow = sb.tile([1, 9], F32, tag="outrow")
            nc.vector.tensor_copy(outrow[:, 1:2], diff_sb[:])
            nc.vector.tensor_copy(outrow[:, 2:3], eff_sb[:])
            nc.vector.tensor_copy(outrow[:, 3:4], tv_sb[:])
            nc.scalar.copy(outrow[:, 5:9], mit_ps[:])
            nc.vector.tensor_reduce(outrow[:, 4:5], outrow[:, 5:9], AX.X,
                                    OP.min)
            nc.vector.tensor_tensor(outrow[:, 0:1], outrow[:, 4:5], tanhd[:],
                                    OP.add)
            nc.sync.dma_start(out=out[:], in_=outrow[:])
            if debug:
                nc.sync.dma_start(out=dbg_gmm[:], in_=gmm[:])
                nc.sync.dma_start(out=dbg_rmat[:], in_=stSb[:, 0:J])
                nc.sync.dma_start(out=dbg_bin[:], in_=binint[:, 0:16])
                nc.sync.dma_start(out=dbg_gj[:], in_=gj4[:])
                nc.sync.dma_start(out=dbg_mm6[:], in_=mmq[:])

    nc.compile()
    return nc


def _get_nc(debug=False):
    key = ("ncd" if debug else "nc")
    if key not in _CACHE:
        _CACHE[key] = _build(debug)
    return _CACHE[key]


def kernel(state, state_memory, state_history, partitions, sample_idx,
           trace=False, debug=False):
    global LAST_RESULTS
    state = np.asarray(state, np.float32)
    state_memory = np.asarray(state_memory, np.float32)
    state_history = np.asarray(state_history, np.float32)
    partitions = np.asarray(partitions)
    sample_idx = np.asarray(sample_idx)

    pf = partitions.astype(np.float32)
    mmat = np.empty((D, J), np.float32)
    invc8 = np.empty((J,), np.float32)
    for p in range(NPAIR):
        mmat[:, 2 * p] = pf[p]
        mmat[:, 2 * p + 1] = np.float32(1.0) - pf[p]
        invc8[2 * p] = np.float32(1.0) / pf[p].sum(dtype=np.float32)
        invc8[2 * p + 1] = np.float32(1.0) / (np.float32(1.0) - pf[p]).sum(
            dtype=np.float32)
    invc = np.zeros((128, 1), np.float32)
    for q in range(3):
        invc[32 * q:32 * q + J, 0] = invc8

    # SBUF-layout pre-swizzles: [p, k*F + f] = src[k*128 + p, f]
    def swz(src_dxf):
        f = src_dxf.shape[1]
        return np.ascontiguousarray(
            src_dxf.reshape(NDC, 128, f).transpose(1, 0, 2).reshape(
                128, NDC * f))

    mmatb = swz(mmat).astype(np.float16)
    memory = np.concatenate([state, state_memory[state.shape[0]:]], axis=0)
    memt = swz(np.ascontiguousarray(memory.T))
    sampt = swz(np.ascontiguousarray(memory[sample_idx].T))

    sel = np.zeros((128, 3 * J), np.float32)
    idrep = np.zeros((128, J), np.float32)
    for q in range(3):
        for j in range(J):
            sel[32 * q + j, q * J + j] = 1.0
            idrep[32 * q + j, j] = 1.0
    idrepT = np.ascontiguousarray(idrep.T)
    ident = np.eye(NB, dtype=np.float32)
    bd440 = np.zeros((NPAIR, NPAIR * NB), np.float32)
    bd404 = np.zeros((NPAIR * NB, NPAIR), np.float32)
    for p in range(NPAIR):
        bd440[p, p * NB:(p + 1) * NB] = 1.0
        bd404[p * NB:(p + 1) * NB, p] = 1.0

    in_maps = []
    for c in range(N_CORES):
        htc = np.ascontiguousarray(state_history[c * TL:(c + 1) * TL, :].T)
        in_maps.append({"ht": htc, "mmat": mmatb, "invc": invc,
                        "memt": memt, "sampt": sampt, "sel": sel,
                        "idrep": idrep, "idrepT": idrepT, "ident": ident,
                        "bd440": bd440,
                        "bd404": bd404})

    nc = _get_nc(debug)
    res = run_bass_kernel_spmd(nc, in_maps, list(range(N_CORES)),
                               trace=trace)
    LAST_RESULTS = res
    return np.asarray(res.results[0]["out"], np.float32)


# revision 34
# speedup vs baseline: 1.0165x; 1.0105x over previous
"""Trainium2 Bass kernel for nn_ConsciousnessMonitor (histogram_binning).

kernel(**inputs) takes FULL unsharded numpy inputs, returns the full (9,)
float32 output. Shards state_history along time across 8 NeuronCores.

Per core: stream HT [2048, 4096] with an f32->fp16 cast in the SWDGE DMA
(halves SBUF-side traffic; fp16 keeps bin-edge jitter ~1e-3) and
accumulate the 8 masked-mean series on the PE (fp16, 1 cycle/row) into 3
quadrant-packed PSUM banks. Raw-sum min/max per bank (full 128-lane DVE
reduces + PE one-hot lane gather), AllReduce(max) of [max|-min], then a
per-partition affine (scale/bias replicated across quadrants via one PE
matmul), PE transposes into t-major layout, fused clamp+int cast, fp16
one-hot (DVE 2x) and fp16 PE joint histograms packed into one PSUM bank.
One DMA ships all four joints to the AllReduce(add); MI for all 4 pairs
is computed vectorized in a [40, 10] pairs-on-partitions layout using
host-supplied block-diagonal constants. sqrt/tanh are computed via
exp/ln only, so a single activation-table reload pair happens inside the
AllReduce window (none on the critical path). The differentiation branch
(Gram matrix, variance, cdist) overlaps the stream and the collective
windows.

Self-contained: shapes/sharding hardcoded; reads no sibling files.
"""
import numpy as np

import concourse.bacc as bacc
import concourse.tile as tile
import concourse.mybir as mybir
from concourse.bass_utils import run_bass_kernel_spmd

F32 = mybir.dt.float32
F16 = mybir.dt.float16
I32 = mybir.dt.int32
AX = mybir.AxisListType
OP = mybir.AluOpType
ACT = mybir.ActivationFunctionType

N_CORES = 8
T, D = 32768, 2048
TL = T // N_CORES          # 4096 time steps per core
NB = 10                    # histogram bins per axis
NPAIR = 4                  # partitions (mask pairs)
J = 2 * NPAIR              # 8 masked-mean columns
NTC = TL // 512            # 8 accumulator groups (512 t each)
NDC = D // 128             # 16 contraction chunks
NCH = TL // 128            # 32 binning chunks of 128 t
MEM = 100
SN = 10

# accumulator tcn -> (bank b, quadrant q): tcn = 3*b + q, q in {0,1,2}
ACC_MAP = [(tcn // 3, tcn % 3) for tcn in range(NTC)]

_CACHE = {}
LAST_RESULTS = None


def _build(debug=False, variant="main"):
    sim1 = variant.startswith("sim1")
    nc = bacc.Bacc("TRN2", target_bir_lowering=False, debug=False,
                   num_devices=1 if sim1 else N_CORES)
    ht = nc.dram_tensor("ht", [D, TL], F32, kind="ExternalInput").ap()
    mmat = nc.dram_tensor("mmat", [128, NDC * J], F16,
                          kind="ExternalInput").ap()
    invc = nc.dram_tensor("invc", [128, 1], F32, kind="ExternalInput").ap()
    memt = nc.dram_tensor("memt", [128, NDC * MEM], F16,
                          kind="ExternalInput").ap()
    sampt = nc.dram_tensor("sampt", [128, NDC * SN], F32,
                           kind="ExternalInput").ap()
    sel = nc.dram_tensor("sel", [128, 3 * J], F32, kind="ExternalInput").ap()
    idrep = nc.dram_tensor("idrep", [128, J], F32, kind="ExternalInput").ap()
    idrepT = nc.dram_tensor("idrepT", [J, 128], F32,
                            kind="ExternalInput").ap()
    ident = nc.dram_tensor("ident", [NB, NB], F32, kind="ExternalInput").ap()
    bd440 = nc.dram_tensor("bd440", [NPAIR, NPAIR * NB], F32,
                           kind="ExternalInput").ap()
    bd404 = nc.dram_tensor("bd404", [NPAIR * NB, NPAIR], F32,
                           kind="ExternalInput").ap()
    out = nc.dram_tensor("out", [9], F32, kind="ExternalOutput").ap()
    if debug:
        dbg_gmm = nc.dram_tensor("dbg_gmm", [J, 2], F32,
                                 kind="ExternalOutput").ap()
        dbg_rmat = nc.dram_tensor("dbg_rmat", [128, J], F32,
                                  kind="ExternalOutput").ap()
        dbg_bin = nc.dram_tensor("dbg_bin", [128, 16], I32,
                                 kind="ExternalOutput").ap()
        dbg_gj = nc.dram_tensor("dbg_gj", [NPAIR * NB, NB], F32,
                                kind="ExternalOutput").ap()
        dbg_mm6 = nc.dram_tensor("dbg_mm6", [J, 6], F32,
                                 kind="ExternalOutput").ap()

    rg = [list(range(N_CORES))]

    with tile.TileContext(nc) as tc:
        with tc.tile_pool(name="consts", bufs=1) as consts, \
             tc.tile_pool(name="sb", bufs=1) as sb, \
             tc.tile_pool(name="htp", bufs=4) as htp, \
             tc.tile_pool(name="psA", bufs=3, space="PSUM") as psA_pool, \
             tc.tile_pool(name="psJ", bufs=2, space="PSUM") as psJ_pool, \
             tc.tile_pool(name="misc", bufs=3, space="PSUM") as misc, \
             tc.tile_pool(name="dram", bufs=1, space="DRAM") as dram:

            # ---- on-chip constants (DVE; keep Pool queue free for stream) --
            ones128 = consts.tile([128, 1], F32, tag="o128")
            nc.vector.memset(ones128[:], 1.0)
            ones10 = consts.tile([NB, 1], F32, tag="o10")
            nc.vector.memset(ones10[:], 1.0)
            ones1_10 = consts.tile([1, NB], F32, tag="o110")
            nc.vector.memset(ones1_10[:], 1.0)
            invc10 = sb.tile([128, 1], F32, tag="invc10")
            # pin the {ln, exp, copy} activation table before any ACT op
            lnscr = sb.tile([1, 1], F32, tag="lnscr")
            nc.scalar.activation(lnscr[:], ones128[0:1, :], ACT.Ln)

            # ---- early input loads (HWDGE; host pre-swizzled layouts) ----
            m_sb = consts.tile([128, NDC * J], F16, tag="msb")
            nc.sync.dma_start(out=m_sb[:], in_=mmat[:])
            invc_sb = consts.tile([128, 1], F32, tag="invc")
            nc.sync.dma_start(out=invc_sb[:], in_=invc[:])
            nc.vector.tensor_scalar(invc10[:], invc_sb[:], 10.0, None,
                                    OP.mult)
            samp_sb = consts.tile([128, NDC * SN], F32, tag="sampsb")
            nc.sync.dma_start(out=samp_sb[:], in_=sampt[:])

            # ---- stage A: stream HT (f32->fp16 cast DMA), S.T = M.T @ HT --
            psA = [psA_pool.tile([128, 512], F32, tag="sacc", name=f"psA{i}")
                   for i in range(3)]
            # clear stale PSUM rows (gather/reduce read all 128 lanes)
            for b in range(3):
                nc.vector.memset(psA[b][:], 0.0)
            for dk in range(NDC):
                htt = htp.tile([128, TL], F16, tag="htt", name="htt")
                nc.gpsimd.dma_start(out=htt[:],
                                    in_=ht[dk * 128:(dk + 1) * 128, :])
                for tcn in range(NTC):
                    b, q = ACC_MAP[tcn]
                    nc.tensor.matmul(psA[b][32 * q:32 * q + J, :],
                                     m_sb[:, dk * J:(dk + 1) * J],
                                     htt[:, tcn * 512:(tcn + 1) * 512],
                                     start=(dk == 0), stop=(dk == NDC - 1))

            # ---- remaining small loads ----
            mem_sb = consts.tile([128, NDC * MEM], F16, tag="memsb")
            nc.sync.dma_start(out=mem_sb[:], in_=memt[:])
            ident10 = consts.tile([NB, NB], F32, tag="id10")
            nc.sync.dma_start(out=ident10[:], in_=ident[:])
            sel_sb = consts.tile([128, 3 * J], F32, tag="selsb")
            nc.sync.dma_start(out=sel_sb[:], in_=sel[:])
            idrep_sb = consts.tile([128, J], F32, tag="idrepsb")
            nc.sync.dma_start(out=idrep_sb[:], in_=idrep[:])
            idrepT_sb = consts.tile([J, 128], F32, tag="idrepTsb")
            nc.sync.dma_start(out=idrepT_sb[:], in_=idrepT[:])
            bd440_sb = consts.tile([NPAIR, NPAIR * NB], F32, tag="bd440sb")
            nc.sync.dma_start(out=bd440_sb[:], in_=bd440[:])
            bd404_sb = consts.tile([NPAIR * NB, NPAIR], F32, tag="bd404sb")
            nc.sync.dma_start(out=bd404_sb[:], in_=bd404[:])

            # ---- differentiation branch (overlaps stream) ----
            psG = misc.tile([SN, SN], F32, tag="m", name="psG")
            for k in range(NDC):
                nc.tensor.matmul(psG[:], samp_sb[:, k * SN:(k + 1) * SN],
                                 samp_sb[:, k * SN:(k + 1) * SN],
                                 start=(k == 0), stop=(k == NDC - 1))
            sqs = sb.tile([128, NDC * SN], F32, tag="sqs")
            nc.vector.tensor_tensor(sqs[:], samp_sb[:], samp_sb[:], OP.mult)
            psr = misc.tile([SN, 1], F32, tag="m", name="psr")
            for k in range(NDC):
                nc.tensor.matmul(psr[:], sqs[:, k * SN:(k + 1) * SN],
                                 ones128[:], start=(k == 0),
                                 stop=(k == NDC - 1))
            g_sb = sb.tile([SN, SN], F32, tag="gsb")
            nc.scalar.copy(g_sb[:], psG[:])
            r_sb = sb.tile([SN, 1], F32, tag="rsb")
            nc.scalar.copy(r_sb[:], psr[:])

            # variance branch (DVE; overlaps stream)
            mem3 = mem_sb[:].rearrange("p (k f) -> p k f", f=MEM)
            mean16 = sb.tile([128, NDC], F32, tag="mean16")
            nc.vector.tensor_reduce(mean16[:], mem3, AX.X, OP.add)
            nc.vector.tensor_scalar(mean16[:], mean16[:], 1.0 / MEM, None,
                                    OP.mult)
            cent = sb.tile([128, NDC * MEM], F32, tag="cent")
            nc.vector.tensor_tensor(
                cent[:].rearrange("p (k f) -> p k f", f=MEM), mem3,
                mean16[:, :, None].broadcast_to([128, NDC, MEM]), OP.subtract)
            nc.vector.tensor_tensor(cent[:], cent[:], cent[:], OP.mult)
            var16 = sb.tile([128, NDC], F32, tag="var16")
            nc.vector.tensor_reduce(
                var16[:], cent[:].rearrange("p (k f) -> p k f", f=MEM),
                AX.X, OP.add)
            nc.vector.tensor_scalar(var16[:], var16[:], 1.0 / (MEM - 1), None,
                                    OP.mult)
            redv = sb.tile([128, 1], F32, tag="redv")
            nc.vector.tensor_reduce(redv[:], var16[:], AX.X, OP.add)
            v2 = sb.tile([128, NDC], F32, tag="v2")
            nc.vector.tensor_tensor(v2[:], var16[:], var16[:], OP.mult)
            redv2 = sb.tile([128, 1], F32, tag="redv2")
            nc.vector.tensor_reduce(redv2[:], v2[:], AX.X, OP.add)
            pstv = misc.tile([1, 1], F32, tag="m", name="pstv")
            nc.tensor.matmul(pstv[:], redv[:], ones128[:], start=True,
                             stop=True)
            tv_sb = sb.tile([1, 1], F32, tag="tvsb")
            nc.scalar.copy(tv_sb[:], pstv[:])
            pss2 = misc.tile([1, 1], F32, tag="m", name="pss2")
            nc.tensor.matmul(pss2[:], redv2[:], ones128[:], start=True,
                             stop=True)
            s2_sb = sb.tile([1, 1], F32, tag="s2sb")
            nc.scalar.copy(s2_sb[:], pss2[:])

            # cdist pieces that only need PE/copies (overlap stream)
            rrow_ps = misc.tile([1, SN], F32, tag="m", name="rrow_ps")
            nc.tensor.transpose(rrow_ps[:], r_sb[:], ident10[:])
            rrow = sb.tile([1, SN], F32, tag="rrow")
            nc.scalar.copy(rrow[:], rrow_ps[:])
            rB_ps = misc.tile([SN, SN], F32, tag="m", name="rB_ps")
            nc.tensor.matmul(rB_ps[:], ones1_10[:], rrow[:], start=True,
                             stop=True)
            rB = sb.tile([SN, SN], F32, tag="rB")
            nc.scalar.copy(rB[:], rB_ps[:])

            # ---- stage B: raw min/max per bank, PE lane-gather, AllReduce --
            stS = sb.tile([128, 3 * 512], F32, tag="sts")
            mm6 = sb.tile([128, 6], F32, tag="mm6")
            for b in range(3):
                nc.vector.tensor_reduce(mm6[:, b:b + 1], psA[b][:], AX.X,
                                        OP.max)
                nc.vector.tensor_reduce(mm6[:, 3 + b:4 + b], psA[b][:], AX.X,
                                        OP.min)
                nc.scalar.copy(stS[:, b * 512:(b + 1) * 512], psA[b][:])
            psMM = misc.tile([J, 18], F32, tag="m", name="psMM")
            for q in range(3):
                nc.tensor.matmul(psMM[:, q * 6:(q + 1) * 6],
                                 sel_sb[:, q * J:(q + 1) * J], mm6[:],
                                 start=True, stop=True)
            psMMv = psMM[:].rearrange("j (q s) -> j q s", s=6)
            mmq = sb.tile([J, 6], F32, tag="mmq")
            nc.vector.tensor_reduce(mmq[:, 0:3, None],
                                    psMMv[:, :, 0:3].rearrange(
                                        "j q s -> j s q"), AX.X, OP.max)
            nc.vector.tensor_reduce(mmq[:, 3:6, None],
                                    psMMv[:, :, 3:6].rearrange(
                                        "j q s -> j s q"), AX.X, OP.min)
            minmax = sb.tile([J, 2], F32, tag="minmax")
            nc.vector.tensor_reduce(minmax[:, 0:1], mmq[:, 0:3], AX.X,
                                    OP.max)
            tmn = sb.tile([J, 1], F32, tag="tmn")
            nc.vector.tensor_reduce(tmn[:], mmq[:, 3:6], AX.X, OP.min)
            nc.vector.tensor_scalar(minmax[:, 1:2], tmn[:], -1.0, None,
                                    OP.mult)
            cbA = dram.tile([J, 2], F32, tag="cba")
            cbB = dram.tile([J, 2], F32, tag="cbb")
            nc.sync.dma_start(out=cbA[:], in_=minmax[:])
            if sim1:
                nc.sync.dma_start(out=cbB[:], in_=cbA[:])
            else:
                nc.gpsimd.collective_compute("AllReduce", OP.max,
                                             replica_groups=rg,
                                             ins=[cbA.opt()],
                                             outs=[cbB.opt()])
            # ---- differentiation tail (fills the AllReduce windows) ----
            tvsq = sb.tile([1, 1], F32, tag="tvsq")
            nc.vector.tensor_tensor(tvsq[:], tv_sb[:], tv_sb[:], OP.mult)
            dden = sb.tile([1, 1], F32, tag="dden")
            nc.vector.scalar_tensor_tensor(dden[:], tvsq[:], 1e-6, s2_sb[:],
                                           OP.mult, OP.add)
            rdden = sb.tile([1, 1], F32, tag="rdden")
            nc.vector.reciprocal(rdden[:], dden[:])
            eff_sb = sb.tile([1, 1], F32, tag="effsb")
            nc.vector.tensor_tensor(eff_sb[:], tvsq[:], rdden[:], OP.mult)
            d2 = sb.tile([SN, SN], F32, tag="d2")
            nc.vector.scalar_tensor_tensor(d2[:], g_sb[:], -2.0, rB[:],
                                           OP.mult, OP.add)
            nc.vector.tensor_scalar(d2[:], d2[:], r_sb[:], 0.0, OP.add,
                                    OP.max)
            # sqrt(x) = exp(0.5*ln(x)); Ln ops first, then Exp ops (one
            # table switch each way, both inside the AllReduce window)
            lnd2 = sb.tile([SN, SN], F32, tag="lnd2")
            nc.scalar.activation(lnd2[:], d2[:], ACT.Ln)
            lntv = sb.tile([1, 1], F32, tag="lntv")
            nc.scalar.activation(lntv[:], tv_sb[:], ACT.Ln)
            dst = sb.tile([SN, SN], F32, tag="dst")
            nc.scalar.activation(dst[:], lnd2[:], ACT.Exp, scale=0.5)
            sqtv = sb.tile([1, 1], F32, tag="sqtv")
            nc.scalar.activation(sqtv[:], lntv[:], ACT.Exp, scale=0.5)
            dsum = sb.tile([SN, 1], F32, tag="dsum")
            nc.vector.tensor_reduce(dsum[:], dst[:], AX.X, OP.add)
            psD = misc.tile([1, 1], F32, tag="m", name="psD")
            nc.tensor.matmul(psD[:], dsum[:], ones10[:], start=True, stop=True)
            avg_sb = sb.tile([1, 1], F32, tag="avgsb")
            nc.vector.tensor_scalar(avg_sb[:], psD[:],
                                    float(1.0 / (SN * (SN - 1) + 1e-6)), None,
                                    OP.mult)
            diff_sb = sb.tile([1, 1], F32, tag="diffsb")
            nc.vector.tensor_tensor(diff_sb[:], sqtv[:], avg_sb[:], OP.mult)
            # tanh(x) = 1 - 2/(exp(2x)+1)
            e2x = sb.tile([1, 1], F32, tag="e2x")
            nc.scalar.activation(e2x[:], diff_sb[:], ACT.Exp, scale=2.0)
            nc.vector.tensor_scalar(e2x[:], e2x[:], 1.0, None, OP.add)
            re2 = sb.tile([1, 1], F32, tag="re2")
            nc.vector.reciprocal(re2[:], e2x[:])
            tanhd = sb.tile([1, 1], F32, tag="tanhd")
            nc.vector.tensor_scalar(tanhd[:], re2[:], -2.0, 1.0, OP.mult,
                                    OP.add)
            # re-pin the natural-log table before the MI log; reading tanhd
            # chains this after the last Exp so Tile cannot hoist it
            nc.scalar.activation(lnscr[:], tanhd[:], ACT.Abs)
            nc.scalar.activation(lnscr[:], lnscr[:], ACT.Ln)

            gmm = sb.tile([J, 2], F32, tag="gmm")
            nc.sync.dma_start(out=gmm[:], in_=cbB[:])

            # s1' = 10*invc/((max-min)*invc + 1e-6);
            # b1 = -min*s1' - 0.5 (RNE cast -> floor)  [raw-sum domain]
            dden2 = sb.tile([J, 1], F32, tag="dden2")
            nc.vector.tensor_tensor(dden2[:], gmm[:, 0:1], gmm[:, 1:2],
                                    OP.add)
            nc.vector.tensor_scalar(dden2[:], dden2[:], invc_sb[0:J, :], 1e-6,
                                    OP.mult, OP.add)
            rdd = sb.tile([J, 1], F32, tag="rdd")
            nc.vector.reciprocal(rdd[:], dden2[:])
            s1p8 = sb.tile([J, 2], F32, tag="s1p8")
            nc.vector.tensor_tensor(s1p8[:, 0:1], rdd[:], invc10[0:J, :],
                                    OP.mult)
            nc.vector.tensor_scalar(s1p8[:, 1:2], gmm[:, 1:2], s1p8[:, 0:1],
                                    -0.5, OP.mult, OP.add)
            # replicate [s1p | b1] to all 128 partitions via PE
            sb128_ps = misc.tile([128, 2], F32, tag="m", name="sb128")
            nc.tensor.matmul(sb128_ps[:], idrepT_sb[:], s1p8[:], start=True,
                             stop=True)
            sb128 = sb.tile([128, 2], F32, tag="sb128")
            nc.vector.tensor_copy(sb128[:], sb128_ps[:])
            s1p = sb128[:, 0:1]
            b1 = sb128[:, 1:2]

            # ---- stage C: per-partition affine, PE transpose, bin, joints --
            stSb = sb.tile([128, 3 * 512], F32, tag="stsb")
            psC = misc.tile([128, NCH * J], F32, tag="m", name="psC")
            for b in range(3):
                if b == 1:
                    nc.vector.tensor_scalar(stSb[:, b * 512:(b + 1) * 512],
                                            stS[:, b * 512:(b + 1) * 512],
                                            s1p, b1, OP.mult, OP.add)
                else:
                    nc.scalar.activation(stSb[:, b * 512:(b + 1) * 512],
                                         stS[:, b * 512:(b + 1) * 512],
                                         ACT.Identity, bias=b1,
                                         scale=s1p)
                for tcn in range(3 * b, min(3 * b + 3, NTC)):
                    _, q = ACC_MAP[tcn]
                    for c in range(4):
                        gc = tcn * 4 + c
                        nc.tensor.transpose(
                            psC[:, gc * J:(gc + 1) * J],
                            stSb[32 * q:32 * q + J,
                                 b * 512 + c * 128:b * 512 + c * 128 + 128],
                            idrep_sb[32 * q:32 * q + J, :])
            binint = sb.tile([128, NCH * J], I32, tag="binint")
            nc.vector.tensor_scalar(binint[:], psC[:], 0.0, float(NB - 1),
                                    OP.max, OP.min)
            binh = sb.tile([128, NCH * J], F16, tag="binh")
            nc.vector.tensor_copy(binh[:], binint[:])
            # one-hot bin-major: plane b at cols [b*256, (b+1)*256)
            ohsb = sb.tile([128, NB * NCH * J], F16, tag="ohsb")
            for b in range(NB):
                nc.vector.tensor_scalar(
                    ohsb[:, b * NCH * J:(b + 1) * NCH * J], binh[:],
                    float(b), None, OP.is_equal)
            ohb = ohsb[:].rearrange("p (b c) -> p b c", b=NB)
            # joint histograms packed in one PSUM bank: pairs 0-2 at rows
            # 32p cols 0:10, pair 3 at rows 0-9 cols 10:20
            psJt = psJ_pool.tile([128, 2 * NB], F32, tag="pj", name="psJt")
            nc.vector.memset(psJt[:], 0.0)
            jm1 = sb.tile([128, 2 * NB], F32, tag="jm1")
            cbj = dram.tile([96, 2 * NB], F32, tag="cbj")
            cbj2 = dram.tile([96, 2 * NB], F32, tag="cbj2")
            for p in (3, 0, 1, 2):
                outap = (psJt[0:NB, NB:2 * NB] if p == 3
                         else psJt[32 * p:32 * p + NB, 0:NB])
                for c in range(NCH):
                    nc.tensor.matmul(outap,
                                     ohb[:, :, c * J + 2 * p],
                                     ohb[:, :, c * J + 2 * p + 1],
                                     start=(c == 0), stop=(c == NCH - 1))
            nc.scalar.copy(jm1[:], psJt[:])
            nc.sync.dma_start(out=cbj[:], in_=jm1[0:96, :])
            if sim1:
                nc.sync.dma_start(out=cbj2[:], in_=cbj[:])
            else:
                nc.gpsimd.collective_compute("AllReduce", OP.add,
                                             replica_groups=rg,
                                             ins=[cbj.opt()],
                                             outs=[cbj2.opt()])
            # gj4: pairs stacked along partitions, [40, 10]
            gj4 = sb.tile([NPAIR * NB, NB], F32, tag="gj4")
            nc.sync.dma_start(
                out=gj4[0:3 * NB, :],
                in_=cbj2[:].rearrange("(q r) c -> q r c",
                                      r=32)[0:3, 0:NB, 0:NB])
            nc.scalar.dma_start(out=gj4[3 * NB:4 * NB, :],
                                in_=cbj2[0:NB, NB:2 * NB])

            # ---- stage D: MI for all 4 pairs at once ----
            rowsum = sb.tile([NPAIR * NB, 1], F32, tag="rowsum")
            nc.vector.tensor_reduce(rowsum[:], gj4[:], AX.X, OP.add)
            colps = misc.tile([NPAIR, NB], F32, tag="m", name="colps")
            nc.tensor.matmul(colps[:], bd404_sb[:], gj4[:], start=True,
                             stop=True)
            tot4 = sb.tile([NPAIR, 1], F32, tag="tot4")
            nc.vector.tensor_reduce(tot4[:], colps[:], AX.X, OP.add)
            nc.vector.tensor_scalar(tot4[:], tot4[:], 1e-10, None, OP.add)
            tinv = sb.tile([NPAIR, 1], F32, tag="tinv")
            nc.vector.reciprocal(tinv[:], tot4[:])
            t40_ps = misc.tile([NPAIR * NB, 1], F32, tag="m", name="t40_ps")
            nc.tensor.matmul(t40_ps[:], bd440_sb[:], tinv[:], start=True,
                             stop=True)
            t40 = sb.tile([NPAIR * NB, 1], F32, tag="t40")
            nc.scalar.copy(t40[:], t40_ps[:])
            pyn = sb.tile([NPAIR, NB], F32, tag="pyn")
            nc.vector.tensor_scalar(pyn[:], colps[:], tinv[:], None, OP.mult)
            pyB = misc.tile([NPAIR * NB, NB], F32, tag="m", name="pyB")
            nc.tensor.matmul(pyB[:], bd440_sb[:], pyn[:], start=True,
                             stop=True)
            px = sb.tile([NPAIR * NB, 1], F32, tag="px")
            nc.vector.tensor_tensor(px[:], rowsum[:], t40[:], OP.mult)
            jn = sb.tile([NPAIR * NB, NB], F32, tag="jn")
            nc.vector.tensor_scalar(jn[:], gj4[:], t40[:], None, OP.mult)
            num = sb.tile([NPAIR * NB, NB], F32, tag="num")
            nc.vector.tensor_scalar(num[:], jn[:], 1e-10, None, OP.add)
            outer = sb.tile([NPAIR * NB, NB], F32, tag="outer")
            nc.vector.tensor_scalar(outer[:], pyB[:], px[:], 1e-10, OP.mult,
                                    OP.add)
            rout = sb.tile([NPAIR * NB, NB], F32, tag="rout")
            nc.vector.reciprocal(rout[:], outer[:])
            nc.vector.tensor_tensor(num[:], num[:], rout[:], OP.mult)
            lg = sb.tile([NPAIR * NB, NB], F32, tag="lg")
            nc.scalar.activation(lg[:], num[:], ACT.Ln)
            nc.vector.tensor_tensor(lg[:], jn[:], lg[:], OP.mult)
            ms = sb.tile([NPAIR * NB, 1], F32, tag="ms")
            nc.vector.tensor_reduce(ms[:], lg[:], AX.X, OP.add)
            mi4_ps = misc.tile([NPAIR, 1], F32, tag="m", name="mi4_ps")
            nc.tensor.matmul(mi4_ps[:], bd404_sb[:], ms[:], start=True,
                             stop=True)
            mi4 = sb.tile([NPAIR, 1], F32, tag="mi4")
            nc.vector.tensor_scalar(mi4[:], mi4_ps[:], 0.0, None, OP.max)
            mit_ps = misc.tile([1, NPAIR], F32, tag="m", name="mit_ps")
            nc.tensor.transpose(mit_ps[:], mi4[:], ident10[0:NPAIR, 0:NPAIR])
            outrow = sb.tile([1, 9], F32, tag="outrow")
            nc.vector.tensor_copy(outrow[:, 1:2], diff_sb[:])
            nc.vector.tensor_copy(outrow[:, 2:3], eff_sb[:])
            nc.vector.tensor_copy(outrow[:, 3:4], tv_sb[:])
            nc.scalar.copy(outrow[:, 5:9], mit_ps[:])
            nc.vector.tensor_reduce(outrow[:, 4:5], outrow[:, 5:9], AX.X,
                                    OP.min)
            nc.vector.tensor_tensor(outrow[:, 0:1], outrow[:, 4:5], tanhd[:],
                                    OP.add)
            nc.sync.dma_start(out=out[:], in_=outrow[:])
            if debug:
                nc.sync.dma_start(out=dbg_gmm[:], in_=gmm[:])
                nc.sync.dma_start(out=dbg_rmat[:], in_=stSb[:, 0:J])
                nc.sync.dma_start(out=dbg_bin[:], in_=binint[:, 0:16])
                nc.sync.dma_start(out=dbg_gj[:], in_=gj4[:])
                nc.sync.dma_start(out=dbg_mm6[:], in_=mmq[:])

    nc.compile()
    return nc


def _get_nc(debug=False):
    key = ("ncd" if debug else "nc")
    if key not in _CACHE:
        _CACHE[key] = _build(debug)
    return _CACHE[key]


def kernel(state, state_memory, state_history, partitions, sample_idx,
           trace=False, debug=False):
    global LAST_RESULTS
    state = np.asarray(state, np.float32)
    state_memory = np.asarray(state_memory, np.float32)
    state_history = np.asarray(state_history, np.float32)
    partitions = np.asarray(partitions)
    sample_idx = np.asarray(sample_idx)

    pf = partitions.astype(np.float32)
    mmat = np.empty((D, J), np.float32)
    invc8 = np.empty((J,), np.float32)
    for p in range(NPAIR):
        mmat[:, 2 * p] = pf[p]
        mmat[:, 2 * p + 1] = np.float32(1.0) - pf[p]
        invc8[2 * p] = np.float32(1.0) / pf[p].sum(dtype=np.float32)
        invc8[2 * p + 1] = np.float32(1.0) / (np.float32(1.0) - pf[p]).sum(
            dtype=np.float32)
    invc = np.zeros((128, 1), np.float32)
    for q in range(3):
        invc[32 * q:32 * q + J, 0] = invc8

    # SBUF-layout pre-swizzles: [p, k*F + f] = src[k*128 + p, f]
    def swz(src_dxf):
        f = src_dxf.shape[1]
        return np.ascontiguousarray(
            src_dxf.reshape(NDC, 128, f).transpose(1, 0, 2).reshape(
                128, NDC * f))

    mmatb = swz(mmat).astype(np.float16)
    memory = np.concatenate([state, state_memory[state.shape[0]:]], axis=0)
    memt = swz(np.ascontiguousarray(memory.T)).astype(np.float16)
    sampt = swz(np.ascontiguousarray(memory[sample_idx].T))

    sel = np.zeros((128, 3 * J), np.float32)
    idrep = np.zeros((128, J), np.float32)
    for q in range(3):
        for j in range(J):
            sel[32 * q + j, q * J + j] = 1.0
            idrep[32 * q + j, j] = 1.0
    idrepT = np.ascontiguousarray(idrep.T)
    ident = np.eye(NB, dtype=np.float32)
    bd440 = np.zeros((NPAIR, NPAIR * NB), np.float32)
    bd404 = np.zeros((NPAIR * NB, NPAIR), np.float32)
    for p in range(NPAIR):
        bd440[p, p * NB:(p + 1) * NB] = 1.0
        bd404[p * NB:(p + 1) * NB, p] = 1.0

    in_maps = []
    for c in range(N_CORES):
        htc = np.ascontiguousarray(state_history[c * TL:(c + 1) * TL, :].T)
        in_maps.append({"ht": htc, "mmat": mmatb, "invc": invc,
                        "memt": memt, "sampt": sampt, "sel": sel,
                        "idrep": idrep, "idrepT": idrepT, "ident": ident,
                        "bd440": bd440,
                        "bd404": bd404})

    nc = _get_nc(debug)
    res = run_bass_kernel_spmd(nc, in_maps, list(range(N_CORES)),
                               trace=trace)
    LAST_RESULTS = res
    return np.asarray(res.results[0]["out"], np.float32)


# revision 35
# speedup vs baseline: 1.0183x; 1.0018x over previous
"""Trainium2 Bass kernel for nn_ConsciousnessMonitor (histogram_binning).

kernel(**inputs) takes FULL unsharded numpy inputs, returns the full (9,)
float32 output. Shards state_history along time across 8 NeuronCores.

Per core: stream HT [2048, 4096] with an f32->fp16 cast in the SWDGE DMA
(halves SBUF-side traffic; fp16 keeps bin-edge jitter ~1e-3) and
accumulate the 8 masked-mean series on the PE (fp16, 1 cycle/row) into 3
quadrant-packed PSUM banks. Raw-sum min/max per bank (full 128-lane DVE
reduces + PE one-hot lane gather), AllReduce(max) of [max|-min], then a
per-partition affine (scale/bias replicated across quadrants via one PE
matmul), PE transposes into t-major layout, fused clamp+int cast, fp16
one-hot (DVE 2x) and fp16 PE joint histograms packed into one PSUM bank.
One DMA ships all four joints to the AllReduce(add); MI for all 4 pairs
is computed vectorized in a [40, 10] pairs-on-partitions layout using
host-supplied block-diagonal constants. sqrt/tanh are computed via
exp/ln only, so a single activation-table reload pair happens inside the
AllReduce window (none on the critical path). The differentiation branch
(Gram matrix, variance, cdist) overlaps the stream and the collective
windows.

Self-contained: shapes/sharding hardcoded; reads no sibling files.
"""
import numpy as np

import concourse.bacc as bacc
import concourse.tile as tile
import concourse.mybir as mybir
from concourse.bass_utils import run_bass_kernel_spmd

F32 = mybir.dt.float32
F16 = mybir.dt.float16
I32 = mybir.dt.int32
AX = mybir.AxisListType
OP = mybir.AluOpType
ACT = mybir.ActivationFunctionType

N_CORES = 8
T, D = 32768, 2048
TL = T // N_CORES          # 4096 time steps per core
NB = 10                    # histogram bins per axis
NPAIR = 4                  # partitions (mask pairs)
J = 2 * NPAIR              # 8 masked-mean columns
NTC = TL // 512            # 8 accumulator groups (512 t each)
NDC = D // 128             # 16 contraction chunks
NCH = TL // 128            # 32 binning chunks of 128 t
MEM = 100
SN = 10

# accumulator tcn -> (bank b, quadrant q): tcn = 3*b + q, q in {0,1,2}
ACC_MAP = [(tcn // 3, tcn % 3) for tcn in range(NTC)]

_CACHE = {}
LAST_RESULTS = None


def _build(debug=False, variant="main"):
    sim1 = variant.startswith("sim1")
    nc = bacc.Bacc("TRN2", target_bir_lowering=False, debug=False,
                   num_devices=1 if sim1 else N_CORES)
    ht = nc.dram_tensor("ht", [D, TL], F32, kind="ExternalInput").ap()
    mmat = nc.dram_tensor("mmat", [128, NDC * J], F16,
                          kind="ExternalInput").ap()
    invc = nc.dram_tensor("invc", [128, 1], F32, kind="ExternalInput").ap()
    memt = nc.dram_tensor("memt", [128, NDC * MEM], F16,
                          kind="ExternalInput").ap()
    sampt = nc.dram_tensor("sampt", [128, NDC * SN], F16,
                           kind="ExternalInput").ap()
    sel = nc.dram_tensor("sel", [128, 3 * J], F32, kind="ExternalInput").ap()
    idrep = nc.dram_tensor("idrep", [128, J], F32, kind="ExternalInput").ap()
    idrepT = nc.dram_tensor("idrepT", [J, 128], F32,
                            kind="ExternalInput").ap()
    ident = nc.dram_tensor("ident", [NB, NB], F32, kind="ExternalInput").ap()
    bd440 = nc.dram_tensor("bd440", [NPAIR, NPAIR * NB], F32,
                           kind="ExternalInput").ap()
    bd404 = nc.dram_tensor("bd404", [NPAIR * NB, NPAIR], F32,
                           kind="ExternalInput").ap()
    out = nc.dram_tensor("out", [9], F32, kind="ExternalOutput").ap()
    if debug:
        dbg_gmm = nc.dram_tensor("dbg_gmm", [J, 2], F32,
                                 kind="ExternalOutput").ap()
        dbg_rmat = nc.dram_tensor("dbg_rmat", [128, J], F32,
                                  kind="ExternalOutput").ap()
        dbg_bin = nc.dram_tensor("dbg_bin", [128, 16], I32,
                                 kind="ExternalOutput").ap()
        dbg_gj = nc.dram_tensor("dbg_gj", [NPAIR * NB, NB], F32,
                                kind="ExternalOutput").ap()
        dbg_mm6 = nc.dram_tensor("dbg_mm6", [J, 6], F32,
                                 kind="ExternalOutput").ap()

    rg = [list(range(N_CORES))]

    with tile.TileContext(nc) as tc:
        with tc.tile_pool(name="consts", bufs=1) as consts, \
             tc.tile_pool(name="sb", bufs=1) as sb, \
             tc.tile_pool(name="htp", bufs=4) as htp, \
             tc.tile_pool(name="psA", bufs=3, space="PSUM") as psA_pool, \
             tc.tile_pool(name="psJ", bufs=2, space="PSUM") as psJ_pool, \
             tc.tile_pool(name="misc", bufs=3, space="PSUM") as misc, \
             tc.tile_pool(name="dram", bufs=1, space="DRAM") as dram:

            # ---- on-chip constants (DVE; keep Pool queue free for stream) --
            ones128 = consts.tile([128, 1], F32, tag="o128")
            nc.vector.memset(ones128[:], 1.0)
            ones10 = consts.tile([NB, 1], F32, tag="o10")
            nc.vector.memset(ones10[:], 1.0)
            ones1_10 = consts.tile([1, NB], F32, tag="o110")
            nc.vector.memset(ones1_10[:], 1.0)
            invc10 = sb.tile([128, 1], F32, tag="invc10")
            # pin the {ln, exp, copy} activation table before any ACT op
            lnscr = sb.tile([1, 1], F32, tag="lnscr")
            nc.scalar.activation(lnscr[:], ones128[0:1, :], ACT.Ln)

            # ---- early input loads (HWDGE; host pre-swizzled layouts) ----
            m_sb = consts.tile([128, NDC * J], F16, tag="msb")
            nc.sync.dma_start(out=m_sb[:], in_=mmat[:])
            invc_sb = consts.tile([128, 1], F32, tag="invc")
            nc.sync.dma_start(out=invc_sb[:], in_=invc[:])
            nc.vector.tensor_scalar(invc10[:], invc_sb[:], 10.0, None,
                                    OP.mult)
            samp_sb = consts.tile([128, NDC * SN], F16, tag="sampsb")
            nc.sync.dma_start(out=samp_sb[:], in_=sampt[:])

            # ---- stage A: stream HT (f32->fp16 cast DMA), S.T = M.T @ HT --
            psA = [psA_pool.tile([128, 512], F32, tag="sacc", name=f"psA{i}")
                   for i in range(3)]
            # clear stale PSUM rows (gather/reduce read all 128 lanes)
            for b in range(3):
                nc.vector.memset(psA[b][:], 0.0)
            for dk in range(NDC):
                htt = htp.tile([128, TL], F16, tag="htt", name="htt")
                nc.gpsimd.dma_start(out=htt[:],
                                    in_=ht[dk * 128:(dk + 1) * 128, :])
                for tcn in range(NTC):
                    b, q = ACC_MAP[tcn]
                    nc.tensor.matmul(psA[b][32 * q:32 * q + J, :],
                                     m_sb[:, dk * J:(dk + 1) * J],
                                     htt[:, tcn * 512:(tcn + 1) * 512],
                                     start=(dk == 0), stop=(dk == NDC - 1))

            # ---- remaining small loads ----
            mem_sb = consts.tile([128, NDC * MEM], F16, tag="memsb")
            nc.sync.dma_start(out=mem_sb[:], in_=memt[:])
            ident10 = consts.tile([NB, NB], F32, tag="id10")
            nc.sync.dma_start(out=ident10[:], in_=ident[:])
            sel_sb = consts.tile([128, 3 * J], F32, tag="selsb")
            nc.sync.dma_start(out=sel_sb[:], in_=sel[:])
            idrep_sb = consts.tile([128, J], F32, tag="idrepsb")
            nc.sync.dma_start(out=idrep_sb[:], in_=idrep[:])
            idrepT_sb = consts.tile([J, 128], F32, tag="idrepTsb")
            nc.sync.dma_start(out=idrepT_sb[:], in_=idrepT[:])
            bd440_sb = consts.tile([NPAIR, NPAIR * NB], F32, tag="bd440sb")
            nc.sync.dma_start(out=bd440_sb[:], in_=bd440[:])
            bd404_sb = consts.tile([NPAIR * NB, NPAIR], F32, tag="bd404sb")
            nc.sync.dma_start(out=bd404_sb[:], in_=bd404[:])

            # ---- differentiation branch (overlaps stream) ----
            psG = misc.tile([SN, SN], F32, tag="m", name="psG")
            for k in range(NDC):
                nc.tensor.matmul(psG[:], samp_sb[:, k * SN:(k + 1) * SN],
                                 samp_sb[:, k * SN:(k + 1) * SN],
                                 start=(k == 0), stop=(k == NDC - 1))
            sqs = sb.tile([128, NDC * SN], F32, tag="sqs")
            nc.vector.tensor_tensor(sqs[:], samp_sb[:], samp_sb[:], OP.mult)
            psr = misc.tile([SN, 1], F32, tag="m", name="psr")
            for k in range(NDC):
                nc.tensor.matmul(psr[:], sqs[:, k * SN:(k + 1) * SN],
                                 ones128[:], start=(k == 0),
                                 stop=(k == NDC - 1))
            g_sb = sb.tile([SN, SN], F32, tag="gsb")
            nc.scalar.copy(g_sb[:], psG[:])
            r_sb = sb.tile([SN, 1], F32, tag="rsb")
            nc.scalar.copy(r_sb[:], psr[:])

            # variance branch (DVE; overlaps stream)
            mem3 = mem_sb[:].rearrange("p (k f) -> p k f", f=MEM)
            mean16 = sb.tile([128, NDC], F32, tag="mean16")
            nc.vector.tensor_reduce(mean16[:], mem3, AX.X, OP.add)
            nc.vector.tensor_scalar(mean16[:], mean16[:], 1.0 / MEM, None,
                                    OP.mult)
            cent = sb.tile([128, NDC * MEM], F32, tag="cent")
            nc.vector.tensor_tensor(
                cent[:].rearrange("p (k f) -> p k f", f=MEM), mem3,
                mean16[:, :, None].broadcast_to([128, NDC, MEM]), OP.subtract)
            nc.vector.tensor_tensor(cent[:], cent[:], cent[:], OP.mult)
            var16 = sb.tile([128, NDC], F32, tag="var16")
            nc.vector.tensor_reduce(
                var16[:], cent[:].rearrange("p (k f) -> p k f", f=MEM),
                AX.X, OP.add)
            nc.vector.tensor_scalar(var16[:], var16[:], 1.0 / (MEM - 1), None,
                                    OP.mult)
            redv = sb.tile([128, 1], F32, tag="redv")
            nc.vector.tensor_reduce(redv[:], var16[:], AX.X, OP.add)
            v2 = sb.tile([128, NDC], F32, tag="v2")
            nc.vector.tensor_tensor(v2[:], var16[:], var16[:], OP.mult)
            redv2 = sb.tile([128, 1], F32, tag="redv2")
            nc.vector.tensor_reduce(redv2[:], v2[:], AX.X, OP.add)
            pstv = misc.tile([1, 1], F32, tag="m", name="pstv")
            nc.tensor.matmul(pstv[:], redv[:], ones128[:], start=True,
                             stop=True)
            tv_sb = sb.tile([1, 1], F32, tag="tvsb")
            nc.scalar.copy(tv_sb[:], pstv[:])
            pss2 = misc.tile([1, 1], F32, tag="m", name="pss2")
            nc.tensor.matmul(pss2[:], redv2[:], ones128[:], start=True,
                             stop=True)
            s2_sb = sb.tile([1, 1], F32, tag="s2sb")
            nc.scalar.copy(s2_sb[:], pss2[:])

            # cdist pieces that only need PE/copies (overlap stream)
            rrow_ps = misc.tile([1, SN], F32, tag="m", name="rrow_ps")
            nc.tensor.transpose(rrow_ps[:], r_sb[:], ident10[:])
            rrow = sb.tile([1, SN], F32, tag="rrow")
            nc.scalar.copy(rrow[:], rrow_ps[:])
            rB_ps = misc.tile([SN, SN], F32, tag="m", name="rB_ps")
            nc.tensor.matmul(rB_ps[:], ones1_10[:], rrow[:], start=True,
                             stop=True)
            rB = sb.tile([SN, SN], F32, tag="rB")
            nc.scalar.copy(rB[:], rB_ps[:])

            # ---- stage B: raw min/max per bank, PE lane-gather, AllReduce --
            stS = sb.tile([128, 3 * 512], F32, tag="sts")
            mm6 = sb.tile([128, 6], F32, tag="mm6")
            for b in range(3):
                nc.vector.tensor_reduce(mm6[:, b:b + 1], psA[b][:], AX.X,
                                        OP.max)
                nc.vector.tensor_reduce(mm6[:, 3 + b:4 + b], psA[b][:], AX.X,
                                        OP.min)
                nc.scalar.copy(stS[:, b * 512:(b + 1) * 512], psA[b][:])
            psMM = misc.tile([J, 18], F32, tag="m", name="psMM")
            for q in range(3):
                nc.tensor.matmul(psMM[:, q * 6:(q + 1) * 6],
                                 sel_sb[:, q * J:(q + 1) * J], mm6[:],
                                 start=True, stop=True)
            psMMv = psMM[:].rearrange("j (q s) -> j q s", s=6)
            mmq = sb.tile([J, 6], F32, tag="mmq")
            nc.vector.tensor_reduce(mmq[:, 0:3, None],
                                    psMMv[:, :, 0:3].rearrange(
                                        "j q s -> j s q"), AX.X, OP.max)
            nc.vector.tensor_reduce(mmq[:, 3:6, None],
                                    psMMv[:, :, 3:6].rearrange(
                                        "j q s -> j s q"), AX.X, OP.min)
            minmax = sb.tile([J, 2], F32, tag="minmax")
            nc.vector.tensor_reduce(minmax[:, 0:1], mmq[:, 0:3], AX.X,
                                    OP.max)
            tmn = sb.tile([J, 1], F32, tag="tmn")
            nc.vector.tensor_reduce(tmn[:], mmq[:, 3:6], AX.X, OP.min)
            nc.vector.tensor_scalar(minmax[:, 1:2], tmn[:], -1.0, None,
                                    OP.mult)
            cbA = dram.tile([J, 2], F32, tag="cba")
            cbB = dram.tile([J, 2], F32, tag="cbb")
            nc.sync.dma_start(out=cbA[:], in_=minmax[:])
            if sim1:
                nc.sync.dma_start(out=cbB[:], in_=cbA[:])
            else:
                nc.gpsimd.collective_compute("AllReduce", OP.max,
                                             replica_groups=rg,
                                             ins=[cbA.opt()],
                                             outs=[cbB.opt()])
            # ---- differentiation tail (fills the AllReduce windows) ----
            tvsq = sb.tile([1, 1], F32, tag="tvsq")
            nc.vector.tensor_tensor(tvsq[:], tv_sb[:], tv_sb[:], OP.mult)
            dden = sb.tile([1, 1], F32, tag="dden")
            nc.vector.scalar_tensor_tensor(dden[:], tvsq[:], 1e-6, s2_sb[:],
                                           OP.mult, OP.add)
            rdden = sb.tile([1, 1], F32, tag="rdden")
            nc.vector.reciprocal(rdden[:], dden[:])
            eff_sb = sb.tile([1, 1], F32, tag="effsb")
            nc.vector.tensor_tensor(eff_sb[:], tvsq[:], rdden[:], OP.mult)
            d2 = sb.tile([SN, SN], F32, tag="d2")
            nc.vector.scalar_tensor_tensor(d2[:], g_sb[:], -2.0, rB[:],
                                           OP.mult, OP.add)
            nc.vector.tensor_scalar(d2[:], d2[:], r_sb[:], 0.0, OP.add,
                                    OP.max)
            # sqrt(x) = exp(0.5*ln(x)); Ln ops first, then Exp ops (one
            # table switch each way, both inside the AllReduce window)
            lnd2 = sb.tile([SN, SN], F32, tag="lnd2")
            nc.scalar.activation(lnd2[:], d2[:], ACT.Ln)
            lntv = sb.tile([1, 1], F32, tag="lntv")
            nc.scalar.activation(lntv[:], tv_sb[:], ACT.Ln)
            dst = sb.tile([SN, SN], F32, tag="dst")
            nc.scalar.activation(dst[:], lnd2[:], ACT.Exp, scale=0.5)
            sqtv = sb.tile([1, 1], F32, tag="sqtv")
            nc.scalar.activation(sqtv[:], lntv[:], ACT.Exp, scale=0.5)
            dsum = sb.tile([SN, 1], F32, tag="dsum")
            nc.vector.tensor_reduce(dsum[:], dst[:], AX.X, OP.add)
            psD = misc.tile([1, 1], F32, tag="m", name="psD")
            nc.tensor.matmul(psD[:], dsum[:], ones10[:], start=True, stop=True)
            avg_sb = sb.tile([1, 1], F32, tag="avgsb")
            nc.vector.tensor_scalar(avg_sb[:], psD[:],
                                    float(1.0 / (SN * (SN - 1) + 1e-6)), None,
                                    OP.mult)
            diff_sb = sb.tile([1, 1], F32, tag="diffsb")
            nc.vector.tensor_tensor(diff_sb[:], sqtv[:], avg_sb[:], OP.mult)
            # tanh(x) = 1 - 2/(exp(2x)+1)
            e2x = sb.tile([1, 1], F32, tag="e2x")
            nc.scalar.activation(e2x[:], diff_sb[:], ACT.Exp, scale=2.0)
            nc.vector.tensor_scalar(e2x[:], e2x[:], 1.0, None, OP.add)
            re2 = sb.tile([1, 1], F32, tag="re2")
            nc.vector.reciprocal(re2[:], e2x[:])
            tanhd = sb.tile([1, 1], F32, tag="tanhd")
            nc.vector.tensor_scalar(tanhd[:], re2[:], -2.0, 1.0, OP.mult,
                                    OP.add)
            # re-pin the natural-log table before the MI log; reading tanhd
            # chains this after the last Exp so Tile cannot hoist it
            nc.scalar.activation(lnscr[:], tanhd[:], ACT.Abs)
            nc.scalar.activation(lnscr[:], lnscr[:], ACT.Ln)

            gmm = sb.tile([J, 2], F32, tag="gmm")
            nc.sync.dma_start(out=gmm[:], in_=cbB[:])

            # s1' = 10*invc/((max-min)*invc + 1e-6);
            # b1 = -min*s1' - 0.5 (RNE cast -> floor)  [raw-sum domain]
            dden2 = sb.tile([J, 1], F32, tag="dden2")
            nc.vector.tensor_tensor(dden2[:], gmm[:, 0:1], gmm[:, 1:2],
                                    OP.add)
            nc.vector.tensor_scalar(dden2[:], dden2[:], invc_sb[0:J, :], 1e-6,
                                    OP.mult, OP.add)
            rdd = sb.tile([J, 1], F32, tag="rdd")
            nc.vector.reciprocal(rdd[:], dden2[:])
            s1p8 = sb.tile([J, 2], F32, tag="s1p8")
            nc.vector.tensor_tensor(s1p8[:, 0:1], rdd[:], invc10[0:J, :],
                                    OP.mult)
            nc.vector.tensor_scalar(s1p8[:, 1:2], gmm[:, 1:2], s1p8[:, 0:1],
                                    -0.5, OP.mult, OP.add)
            # replicate [s1p | b1] to all 128 partitions via PE
            sb128_ps = misc.tile([128, 2], F32, tag="m", name="sb128")
            nc.tensor.matmul(sb128_ps[:], idrepT_sb[:], s1p8[:], start=True,
                             stop=True)
            sb128 = sb.tile([128, 2], F32, tag="sb128")
            nc.vector.tensor_copy(sb128[:], sb128_ps[:])
            s1p = sb128[:, 0:1]
            b1 = sb128[:, 1:2]

            # ---- stage C: per-partition affine, PE transpose, bin, joints --
            stSb = sb.tile([128, 3 * 512], F32, tag="stsb")
            psC = misc.tile([128, NCH * J], F32, tag="m", name="psC")
            for b in range(3):
                if b == 1:
                    nc.vector.tensor_scalar(stSb[:, b * 512:(b + 1) * 512],
                                            stS[:, b * 512:(b + 1) * 512],
                                            s1p, b1, OP.mult, OP.add)
                else:
                    nc.scalar.activation(stSb[:, b * 512:(b + 1) * 512],
                                         stS[:, b * 512:(b + 1) * 512],
                                         ACT.Identity, bias=b1,
                                         scale=s1p)
                for tcn in range(3 * b, min(3 * b + 3, NTC)):
                    _, q = ACC_MAP[tcn]
                    for c in range(4):
                        gc = tcn * 4 + c
                        nc.tensor.transpose(
                            psC[:, gc * J:(gc + 1) * J],
                            stSb[32 * q:32 * q + J,
                                 b * 512 + c * 128:b * 512 + c * 128 + 128],
                            idrep_sb[32 * q:32 * q + J, :])
            binint = sb.tile([128, NCH * J], I32, tag="binint")
            nc.vector.tensor_scalar(binint[:], psC[:], 0.0, float(NB - 1),
                                    OP.max, OP.min)
            binh = sb.tile([128, NCH * J], F16, tag="binh")
            nc.vector.tensor_copy(binh[:], binint[:])
            # one-hot bin-major: plane b at cols [b*256, (b+1)*256)
            ohsb = sb.tile([128, NB * NCH * J], F16, tag="ohsb")
            for b in range(NB):
                nc.vector.tensor_scalar(
                    ohsb[:, b * NCH * J:(b + 1) * NCH * J], binh[:],
                    float(b), None, OP.is_equal)
            ohb = ohsb[:].rearrange("p (b c) -> p b c", b=NB)
            # joint histograms packed in one PSUM bank: pairs 0-2 at rows
            # 32p cols 0:10, pair 3 at rows 0-9 cols 10:20
            psJt = psJ_pool.tile([128, 2 * NB], F32, tag="pj", name="psJt")
            nc.vector.memset(psJt[:], 0.0)
            jm1 = sb.tile([128, 2 * NB], F32, tag="jm1")
            cbj = dram.tile([96, 2 * NB], F32, tag="cbj")
            cbj2 = dram.tile([96, 2 * NB], F32, tag="cbj2")
            for p in (3, 0, 1, 2):
                outap = (psJt[0:NB, NB:2 * NB] if p == 3
                         else psJt[32 * p:32 * p + NB, 0:NB])
                for c in range(NCH):
                    nc.tensor.matmul(outap,
                                     ohb[:, :, c * J + 2 * p],
                                     ohb[:, :, c * J + 2 * p + 1],
                                     start=(c == 0), stop=(c == NCH - 1))
            nc.scalar.copy(jm1[:], psJt[:])
            nc.sync.dma_start(out=cbj[:], in_=jm1[0:96, :])
            if sim1:
                nc.sync.dma_start(out=cbj2[:], in_=cbj[:])
            else:
                nc.gpsimd.collective_compute("AllReduce", OP.add,
                                             replica_groups=rg,
                                             ins=[cbj.opt()],
                                             outs=[cbj2.opt()])
            # gj4: pairs stacked along partitions, [40, 10]
            gj4 = sb.tile([NPAIR * NB, NB], F32, tag="gj4")
            nc.sync.dma_start(
                out=gj4[0:3 * NB, :],
                in_=cbj2[:].rearrange("(q r) c -> q r c",
                                      r=32)[0:3, 0:NB, 0:NB])
            nc.gpsimd.dma_start(out=gj4[3 * NB:4 * NB, :],
                                in_=cbj2[0:NB, NB:2 * NB])

            # ---- stage D: MI for all 4 pairs at once ----
            rowsum = sb.tile([NPAIR * NB, 1], F32, tag="rowsum")
            nc.vector.tensor_reduce(rowsum[:], gj4[:], AX.X, OP.add)
            colps = misc.tile([NPAIR, NB], F32, tag="m", name="colps")
            nc.tensor.matmul(colps[:], bd404_sb[:], gj4[:], start=True,
                             stop=True)
            tot4 = sb.tile([NPAIR, 1], F32, tag="tot4")
            nc.vector.tensor_reduce(tot4[:], colps[:], AX.X, OP.add)
            nc.vector.tensor_scalar(tot4[:], tot4[:], 1e-10, None, OP.add)
            tinv = sb.tile([NPAIR, 1], F32, tag="tinv")
            nc.vector.reciprocal(tinv[:], tot4[:])
            t40_ps = misc.tile([NPAIR * NB, 1], F32, tag="m", name="t40_ps")
            nc.tensor.matmul(t40_ps[:], bd440_sb[:], tinv[:], start=True,
                             stop=True)
            t40 = sb.tile([NPAIR * NB, 1], F32, tag="t40")
            nc.scalar.copy(t40[:], t40_ps[:])
            pyn = sb.tile([NPAIR, NB], F32, tag="pyn")
            nc.vector.tensor_scalar(pyn[:], colps[:], tinv[:], None, OP.mult)
            pyB = misc.tile([NPAIR * NB, NB], F32, tag="m", name="pyB")
            nc.tensor.matmul(pyB[:], bd440_sb[:], pyn[:], start=True,
                             stop=True)
            px = sb.tile([NPAIR * NB, 1], F32, tag="px")
            nc.vector.tensor_tensor(px[:], rowsum[:], t40[:], OP.mult)
            jn = sb.tile([NPAIR * NB, NB], F32, tag="jn")
            nc.vector.tensor_scalar(jn[:], gj4[:], t40[:], None, OP.mult)
            num = sb.tile([NPAIR * NB, NB], F32, tag="num")
            nc.vector.tensor_scalar(num[:], jn[:], 1e-10, None, OP.add)
            outer = sb.tile([NPAIR * NB, NB], F32, tag="outer")
            nc.vector.tensor_scalar(outer[:], pyB[:], px[:], 1e-10, OP.mult,
                                    OP.add)
            rout = sb.tile([NPAIR * NB, NB], F32, tag="rout")
            nc.vector.reciprocal(rout[:], outer[:])
            nc.vector.tensor_tensor(num[:], num[:], rout[:], OP.mult)
            lg = sb.tile([NPAIR * NB, NB], F32, tag="lg")
            nc.scalar.activation(lg[:], num[:], ACT.Ln)
            nc.vector.tensor_tensor(lg[:], jn[:], lg[:], OP.mult)
            ms = sb.tile([NPAIR * NB, 1], F32, tag="ms")
            nc.vector.tensor_reduce(ms[:], lg[:], AX.X, OP.add)
            mi4_ps = misc.tile([NPAIR, 1], F32, tag="m", name="mi4_ps")
            nc.tensor.matmul(mi4_ps[:], bd404_sb[:], ms[:], start=True,
                             stop=True)
            mi4 = sb.tile([NPAIR, 1], F32, tag="mi4")
            nc.vector.tensor_scalar(mi4[:], mi4_ps[:], 0.0, None, OP.max)
            mit_ps = misc.tile([1, NPAIR], F32, tag="m", name="mit_ps")
            nc.tensor.transpose(mit_ps[:], mi4[:], ident10[0:NPAIR, 0:NPAIR])
            outrow = sb.tile([1, 9], F32, tag="outrow")
            nc.vector.tensor_copy(outrow[:, 1:2], diff_sb[:])
            nc.vector.tensor_copy(outrow[:, 2:3], eff_sb[:])
            nc.vector.tensor_copy(outrow[:, 3:4], tv_sb[:])
            nc.scalar.copy(outrow[:, 5:9], mit_ps[:])
            nc.vector.tensor_reduce(outrow[:, 4:5], outrow[:, 5:9], AX.X,
                                    OP.min)
            nc.vector.tensor_tensor(outrow[:, 0:1], outrow[:, 4:5], tanhd[:],
                                    OP.add)
            nc.sync.dma_start(out=out[:], in_=outrow[:])
            if debug:
                nc.sync.dma_start(out=dbg_gmm[:], in_=gmm[:])
                nc.sync.dma_start(out=dbg_rmat[:], in_=stSb[:, 0:J])
                nc.sync.dma_start(out=dbg_bin[:], in_=binint[:, 0:16])
                nc.sync.dma_start(out=dbg_gj[:], in_=gj4[:])
                nc.sync.dma_start(out=dbg_mm6[:], in_=mmq[:])

    nc.compile()
    return nc


def _get_nc(debug=False):
    key = ("ncd" if debug else "nc")
    if key not in _CACHE:
        _CACHE[key] = _build(debug)
    return _CACHE[key]


def kernel(state, state_memory, state_history, partitions, sample_idx,
           trace=False, debug=False):
    global LAST_RESULTS
    state = np.asarray(state, np.float32)
    state_memory = np.asarray(state_memory, np.float32)
    state_history = np.asarray(state_history, np.float32)
    partitions = np.asarray(partitions)
    sample_idx = np.asarray(sample_idx)

    pf = partitions.astype(np.float32)
    mmat = np.empty((D, J), np.float32)
    invc8 = np.empty((J,), np.float32)
    for p in range(NPAIR):
        mmat[:, 2 * p] = pf[p]
        mmat[:, 2 * p + 1] = np.float32(1.0) - pf[p]
        invc8[2 * p] = np.float32(1.0) / pf[p].sum(dtype=np.float32)
        invc8[2 * p + 1] = np.float32(1.0) / (np.float32(1.0) - pf[p]).sum(
            dtype=np.float32)
    invc = np.zeros((128, 1), np.float32)
    for q in range(3):
        invc[32 * q:32 * q + J, 0] = invc8

    # SBUF-layout pre-swizzles: [p, k*F + f] = src[k*128 + p, f]
    def swz(src_dxf):
        f = src_dxf.shape[1]
        return np.ascontiguousarray(
            src_dxf.reshape(NDC, 128, f).transpose(1, 0, 2).reshape(
                128, NDC * f))

    mmatb = swz(mmat).astype(np.float16)
    memory = np.concatenate([state, state_memory[state.shape[0]:]], axis=0)
    memt = swz(np.ascontiguousarray(memory.T)).astype(np.float16)
    sampt = swz(np.ascontiguousarray(memory[sample_idx].T)).astype(
        np.float16)

    sel = np.zeros((128, 3 * J), np.float32)
    idrep = np.zeros((128, J), np.float32)
    for q in range(3):
        for j in range(J):
            sel[32 * q + j, q * J + j] = 1.0
            idrep[32 * q + j, j] = 1.0
    idrepT = np.ascontiguousarray(idrep.T)
    ident = np.eye(NB, dtype=np.float32)
    bd440 = np.zeros((NPAIR, NPAIR * NB), np.float32)
    bd404 = np.zeros((NPAIR * NB, NPAIR), np.float32)
    for p in range(NPAIR):
        bd440[p, p * NB:(p + 1) * NB] = 1.0
        bd404[p * NB:(p + 1) * NB, p] = 1.0

    in_maps = []
    for c in range(N_CORES):
        htc = np.ascontiguousarray(state_history[c * TL:(c + 1) * TL, :].T)
        in_maps.append({"ht": htc, "mmat": mmatb, "invc": invc,
                        "memt": memt, "sampt": sampt, "sel": sel,
                        "idrep": idrep, "idrepT": idrepT, "ident": ident,
                        "bd440": bd440,
                        "bd404": bd404})

    nc = _get_nc(debug)
    res = run_bass_kernel_spmd(nc, in_maps, list(range(N_CORES)),
                               trace=trace)
    LAST_RESULTS = res
    return np.asarray(res.results[0]["out"], np.float32)
